# revision 23
# baseline (speedup 1.0000x reference)
"""Deformable conv (B=4, C=256, H=W=64, O=256, K=3, pad=1) on 8 NeuronCores.

Sharding: core = (image b, row-half h): each core computes out[b, :, h*32:(h+1)*32, :].

Per-core device pipeline:
  1. DVE computes gather indices (int16, wrapped-16 layout) and bilinear corner
     weights (f32, per-(pos,tap) scalars) from the raw conv offsets.
  2. GpSimd dma_gathers fp16 channel-pair rows (512ch covering x0,x0+1) from a
     host-transposed xT [HW, C] fp16 image in DRAM: 2 gathers (top/bottom row)
     per (tap, position); 1KB descriptors.
  3. DVE combines the 4 corners with scalar_tensor_tensor FMAs (per-partition
     weight scalars; partition = position).
  4. PE transposes [pos, ch] -> [ch, pos] tiles (fp16 PSUM) and runs the
     O=256 x CK=2304 x P=2048 GEMM in fp16 with fp32 PSUM accumulation.
"""

import numpy as np

B, C, H, W = 4, 256, 64, 64
O, KK = 256, 9
HW = H * W                      # 4096
ROWS_PER_CORE = H // 2          # 32
P_CORE = ROWS_PER_CORE * W      # 2048 output positions per core
N_PB = P_CORE // 128            # 16 chunks (position blocks)
IDX_PER_CHUNK = KK * 128        # 1152
N_IDX = N_PB * IDX_PER_CHUNK    # 18432
N_SLOT = N_PB * KK              # 144
N_CKT = 2 * KK                  # 18 ck-tiles of 128
GEMM_NP = 512                   # positions per GEMM pass
N_GP = P_CORE // GEMM_NP        # 4 GEMM passes
N_CORES = 8

_CACHE = {}


def _build_nc(reps=1, skip=()):
    skip = set(skip)
    import concourse.bacc as bacc
    import concourse.mybir as mybir
    from concourse import library_config
    import bass_rust

    F32, F16, I16, I32 = (mybir.dt.float32, mybir.dt.float16,
                          mybir.dt.int16, mybir.dt.int32)
    AF = mybir.ActivationFunctionType
    AL = mybir.AluOpType

    nc = bacc.Bacc("TRN2")

    # ---- DRAM parameters (per-core inputs) ----
    xt = nc.declare_dram_parameter("xt", [(HW + 1) * C], F16, isOutput=False)
    wt = nc.declare_dram_parameter("wt", [N_CKT, 128, O], F16, isOutput=False)
    offy_w = nc.declare_dram_parameter("offy_w", [128, N_SLOT], F32, isOutput=False)
    offx_w = nc.declare_dram_parameter("offx_w", [128, N_SLOT], F32, isOutput=False)
    basy_w = nc.declare_dram_parameter("basy_w", [128, N_SLOT], F32, isOutput=False)
    basx_w = nc.declare_dram_parameter("basx_w", [128, N_SLOT], F32, isOutput=False)
    offy_g = nc.declare_dram_parameter("offy_g", [128, N_IDX // 16], F32, isOutput=False)
    offx_g = nc.declare_dram_parameter("offx_g", [128, N_IDX // 16], F32, isOutput=False)
    basy_g = nc.declare_dram_parameter("basy_g", [128, N_IDX // 16], F32, isOutput=False)
    basx_g = nc.declare_dram_parameter("basx_g", [128, N_IDX // 16], F32, isOutput=False)
    ident = nc.declare_dram_parameter("ident", [128, 128], F16, isOutput=False)
    out = nc.declare_dram_parameter("out", [O, P_CORE], F32, isOutput=True)

    NG = N_IDX // 16     # 1152 free-dim of g-layout

    from contextlib import ExitStack
    st = ExitStack()
    sb = lambda n, s, d: st.enter_context(nc.sbuf_tensor(n, s, d))
    ps = lambda n, s, d: st.enter_context(nc.psum_tensor(n, s, d))

    # ---- SBUF ----
    idx_top = sb("idx_top", [128, NG], I16)
    idx_bot = sb("idx_bot", [128, NG], I16)
    # w-layout pipeline
    ow_y = sb("ow_y", [128, N_SLOT], F32)
    ow_x = sb("ow_x", [128, N_SLOT], F32)
    bw_y = sb("bw_y", [128, N_SLOT], F32)
    bw_x = sb("bw_x", [128, N_SLOT], F32)
    u0 = sb("u0", [128, N_SLOT], F32)
    u1 = sb("u1", [128, N_SLOT], F32)
    u2 = sb("u2", [128, N_SLOT], F32)
    ui = sb("ui", [128, N_SLOT], I32)
    fy = sb("fy", [128, N_SLOT], F32)
    fx = sb("fx", [128, N_SLOT], F32)
    vy0 = sb("vy0", [128, N_SLOT], F32)
    vy1 = sb("vy1", [128, N_SLOT], F32)
    vx0 = sb("vx0", [128, N_SLOT], F32)
    vx1 = sb("vx1", [128, N_SLOT], F32)
    sel = sb("sel", [128, N_SLOT], F32)
    qy0 = sb("qy0", [128, N_SLOT], F32)
    qy1 = sb("qy1", [128, N_SLOT], F32)
    wlx = sb("wlx", [128, N_SLOT], F32)
    wrx = sb("wrx", [128, N_SLOT], F32)
    wTL = sb("wTL", [128, N_SLOT], F32)
    wTR = sb("wTR", [128, N_SLOT], F32)
    wBL = sb("wBL", [128, N_SLOT], F32)
    wBR = sb("wBR", [128, N_SLOT], F32)
    # transposed cols, full [128, N_CKT, P_CORE] fp16
    cols = sb("cols", [128, N_CKT, P_CORE], F16)
    # weights in SBUF [128, N_CKT, O] fp16
    wt_sb = sb("wt_sb", [128, N_CKT, O], F16)
    id_sb = sb("id_sb", [128, 128], F16)
    out_sb = [sb(f"out_sb{i}", [128, GEMM_NP], F32) for i in range(2)]

    # g-layout coord pipeline: temps are dead once idx_top/idx_bot are
    # written, so they live in a nested stack (top of the SBUF allocation
    # stack) that is closed before the gather buffers are allocated — the
    # allocator reuses the space. Runtime-safe: the first gather write is
    # gated on s_idx, which fires after the last coord-temp read.
    st_coord = ExitStack()
    sbc = lambda n, s, d: st_coord.enter_context(nc.sbuf_tensor(n, s, d))
    og_y = sbc("og_y", [128, NG], F32)
    og_x = sbc("og_x", [128, NG], F32)
    bg_y = sbc("bg_y", [128, NG], F32)
    bg_x = sbc("bg_x", [128, NG], F32)
    t0 = sbc("t0", [128, NG], F32)
    t1 = sbc("t1", [128, NG], F32)
    t2 = sbc("t2", [128, NG], F32)
    ti = sbc("ti", [128, NG], I32)
    s_clip = sbc("s_clip", [128, NG], F32)
    st_coord.close()

    # gather buffers (NBUF-deep): [128, KK, 512] fp16 — reuse coord space
    NBUF = 3
    g_top = [sb(f"g_top{i}", [128, KK, 512], F16) for i in range(NBUF)]
    g_bot = [sb(f"g_bot{i}", [128, KK, 512], F16) for i in range(NBUF)]
    # per-(corner, tap) diagonal weight matrices, double buffered:
    # dbuf[b][:, corner, k, :] = diag(w_corner[:, KK*c+k]) as dense [128,128]
    dbuf = [sb(f"dbuf{i}", [128, 4, KK, 128], F16) for i in range(2)]

    NPST = 3
    ps_t = [ps(f"ps_t{i}", [128, 4, 128], F32) for i in range(NPST)]
    ps_g = [ps(f"ps_g{i}", [128, GEMM_NP], F32) for i in range(2)]

    N_LOADS = 10
    IO_ALL = N_LOADS * 16

    def emit_body():
        s_io = nc.alloc_semaphore("s_io")
        s_idx = nc.alloc_semaphore("s_idx")
        s_wts = nc.alloc_semaphore("s_wts")
        s_g = [nc.alloc_semaphore(f"s_g{i}") for i in range(2 * NBUF)]
        s_dg = nc.alloc_semaphore("s_dg")
        s_dga = nc.alloc_semaphore("s_dga")
        s_tr = nc.alloc_semaphore("s_tr")
        s_cpy = nc.alloc_semaphore("s_cpy")
        s_mm = nc.alloc_semaphore("s_mm")
        s_oc = nc.alloc_semaphore("s_oc")
        s_st = [nc.alloc_semaphore(f"s_st{i}") for i in range(2)]
        blk_cm = nc.Block()
        block = blk_cm.__enter__()

        @block.sync
        def _(sync):
            sync.dma_start(wt_sb[:], wt.rearrange("t c e -> c t e")).then_inc(s_io, 16)
            for src, dst in [(offy_w, ow_y), (offx_w, ow_x), (basy_w, bw_y),
                             (basx_w, bw_x), (offy_g, og_y), (offx_g, og_x),
                             (basy_g, bg_y), (basx_g, bg_x), (ident, id_sb)]:
                sync.dma_start(dst[:], src[:]).then_inc(s_io, 16)
            for g in range(N_GP):
                for ob in range(2):
                    j = 2 * g + ob
                    sync.wait_ge(s_oc, j + 1)
                    sync.dma_start(out[ob * 128:(ob + 1) * 128,
                                       g * GEMM_NP:(g + 1) * GEMM_NP],
                                   out_sb[ob][:]).then_inc(s_st[ob], 16)
            sync.wait_ge(s_st[0], 16 * N_GP)
            sync.wait_ge(s_st[1], 16 * N_GP)

        @block.vector
        def _(vector):
            vector.wait_ge(s_io, IO_ALL)

            def TT(out_, a, b, op):
                vector.tensor_tensor(out_, a, b, op)
                vector.drain()

            def TS(out_, a, s1, s2, op0, op1=None):
                if op1 is None:
                    vector.tensor_scalar(out_, a, s1, None, op0)
                else:
                    vector.tensor_scalar(out_, a, s1, s2, op0, op1)
                vector.drain()

            def CP(out_, a):
                vector.tensor_copy(out_, a)
                vector.drain()

            def STT(out_, a, sc, b, op0, op1):
                vector.scalar_tensor_tensor(out_, a, sc, b, op0, op1)
                vector.drain()

            def floor_to(dst_f, dst_frac, src, tmp_i, tmp_a, tmp_b):
                # dst_f = floor(src); dst_frac = src - floor(src)
                CP(tmp_i[:], src[:])                 # rint cast
                CP(tmp_a[:], tmp_i[:])               # back to f32
                TT(tmp_b[:], tmp_a[:], src[:], AL.is_gt)
                TT(dst_f[:], tmp_a[:], tmp_b[:], AL.subtract)
                if dst_frac is not None:
                    TT(dst_frac[:], src[:], dst_f[:], AL.subtract)

            # ---- idx computation (g layout) ----
            TT(t0[:], og_y[:], bg_y[:], AL.add)
            TS(t0[:], t0[:], -8.0, 72.0, AL.max, AL.min)
            floor_to(t1, None, t0, ti, t2, s_clip)
            TT(t0[:], og_x[:], bg_x[:], AL.add)
            TS(t0[:], t0[:], -8.0, 72.0, AL.max, AL.min)
            floor_to(t2, None, t0, ti, s_clip, og_x)   # og_x clobbered as tmp
            TS(s_clip[:], t2[:], 0.0, 63.0, AL.max, AL.min)
            TS(t0[:], t1[:], 0.0, 63.0, AL.max, AL.min)
            STT(t0[:], t0[:], 64.0, s_clip[:], AL.mult, AL.add)
            CP(idx_top[:], t0[:])
            TS(t0[:], t1[:], 1.0, 0.0, AL.add, AL.max)
            TS(t0[:], t0[:], 63.0, None, AL.min)
            STT(t0[:], t0[:], 64.0, s_clip[:], AL.mult, AL.add)
            vector.tensor_copy(idx_bot[:], t0[:])
            vector.drain().then_inc(s_idx, 1)

            # ---- weight computation (w layout) ----
            TT(u0[:], ow_y[:], bw_y[:], AL.add)
            TS(u0[:], u0[:], -8.0, 72.0, AL.max, AL.min)
            floor_to(u1, fy, u0, ui, u2, vy0)
            TS(vy0[:], u1[:], 0.0, None, AL.is_ge)
            TS(u2[:], u1[:], 63.0, None, AL.is_le)
            TT(vy0[:], vy0[:], u2[:], AL.mult)
            TS(vy1[:], u1[:], -1.0, None, AL.is_ge)
            TS(u2[:], u1[:], 62.0, None, AL.is_le)
            TT(vy1[:], vy1[:], u2[:], AL.mult)

            TT(u0[:], ow_x[:], bw_x[:], AL.add)
            TS(u0[:], u0[:], -8.0, 72.0, AL.max, AL.min)
            floor_to(u1, fx, u0, ui, u2, vx0)
            TS(vx0[:], u1[:], 0.0, None, AL.is_ge)
            TS(u2[:], u1[:], 63.0, None, AL.is_le)
            TT(vx0[:], vx0[:], u2[:], AL.mult)
            TS(vx1[:], u1[:], -1.0, None, AL.is_ge)
            TS(u2[:], u1[:], 62.0, None, AL.is_le)
            TT(vx1[:], vx1[:], u2[:], AL.mult)
            TS(sel[:], u1[:], -1.0, None, AL.is_equal)

            TS(u0[:], fy[:], -1.0, 1.0, AL.mult, AL.add)
            TT(qy0[:], u0[:], vy0[:], AL.mult)
            TT(qy1[:], fy[:], vy1[:], AL.mult)
            TS(u0[:], fx[:], -1.0, 1.0, AL.mult, AL.add)
            TT(u0[:], u0[:], vx0[:], AL.mult)
            TT(u1[:], fx[:], vx1[:], AL.mult)
            TT(u2[:], u1[:], u0[:], AL.subtract)
            TT(u2[:], u2[:], sel[:], AL.mult)
            TT(wlx[:], u0[:], u2[:], AL.add)
            TT(u2[:], u1[:], sel[:], AL.mult)
            TT(wrx[:], u1[:], u2[:], AL.subtract)
            TT(wTL[:], qy0[:], wlx[:], AL.mult)
            TT(wTR[:], qy0[:], wrx[:], AL.mult)
            TT(wBL[:], qy1[:], wlx[:], AL.mult)
            vector.tensor_tensor(wBR[:], qy1[:], wrx[:], AL.mult)
            vector.drain().then_inc(s_wts, 1)

            # ---- diag weight builds (corners TL, TR; Act builds BL, BR) ----
            # dbuf[c%2][:, j, k, :] = diag(w_j[:, KK*c+k]) = id * w (scalar
            # per partition); the PE then applies the corner combine as
            # 4 accumulating matmuls g_tile @ diag per (k, cb) tile.
            for c in range(N_PB):
                if c >= 2:
                    vector.wait_ge(s_tr, 5 * (c - 1))
                d = dbuf[c % 2]
                if "combine" in skip:
                    vector.drain().then_inc(s_dg, 1)
                    continue
                for j2, w_ in ((0, wTL), (1, wTR), (2, wBL)):
                    for k in range(KK):
                        vector.tensor_scalar(d[:, j2, k, :], id_sb[:],
                                             w_[:, KK * c + k:KK * c + k + 1],
                                             None, AL.mult)
                vector.drain().then_inc(s_dg, 1)

        @block.gpsimd
        def _(gpsimd):
            gpsimd.load_library(library_config.mlp)
            gpsimd.wait_ge(s_idx, 1)
            xt_view = xt[:].copy()
            xt_view.ap = bass_rust.VecI64Pair([[C, HW], [1, 2 * C]])
            for c in range(N_PB):
                if c >= NBUF:
                    gpsimd.wait_ge(s_tr, 5 * (c - NBUF + 1))
                i0 = c * 72
                # one dma_gather tops out at 1024 idx -> split 1152 = 1024 + 128
                for (lo, hi, s0, s1) in ((0, 64, 0, 8), (64, 72, 8, 9)):
                    nidx = (hi - lo) * 16
                    if "gather" in skip:
                        gpsimd.sem_inc(s_g[c % NBUF], 16)
                        gpsimd.sem_inc(s_g[NBUF + c % NBUF], 16)
                        continue
                    gpsimd.dma_gather(g_top[c % NBUF][:, s0:s1, :], xt_view,
                                      idx_top[:, i0 + lo:i0 + hi],
                                      nidx, nidx, 2 * C,
                                      elem_step=C).then_inc(s_g[c % NBUF], 16)
                    gpsimd.dma_gather(g_bot[c % NBUF][:, s0:s1, :], xt_view,
                                      idx_bot[:, i0 + lo:i0 + hi],
                                      nidx, nidx, 2 * C,
                                      elem_step=C).then_inc(s_g[NBUF + c % NBUF], 16)

        @block.tensor
        def _(tensor):
            tensor.wait_ge(s_io, IO_ALL)
            j = 0
            for c in range(N_PB):
                tensor.wait_ge(s_dg, c + 1)
                tensor.wait_ge(s_dga, c + 1)
                tensor.wait_ge(s_g[c % NBUF], 32 * (c // NBUF + 1))
                tensor.wait_ge(s_g[NBUF + c % NBUF], 32 * (c // NBUF + 1))
                gt, gb = g_top[c % NBUF], g_bot[c % NBUF]
                d = dbuf[c % 2]
                for k in range(KK):
                    for cb in range(2):
                        t = 2 * k + cb
                        g4, q = t // 4, t % 4
                        jj = c * 5 + g4          # global 4-group index (5 groups/chunk)
                        if "pe" in skip:
                            if q == 0:
                                tensor.sem_inc(s_tr, 1)
                            j += 1
                            continue
                        if q == 0 and jj >= NPST:
                            tensor.wait_ge(s_cpy, jj - NPST + 1)
                        corners = (
                            (gt[:, k, cb * 128:(cb + 1) * 128], d[:, 0, k, :]),
                            (gt[:, k, 256 + cb * 128:256 + (cb + 1) * 128],
                             d[:, 1, k, :]),
                            (gb[:, k, cb * 128:(cb + 1) * 128], d[:, 2, k, :]),
                            (gb[:, k, 256 + cb * 128:256 + (cb + 1) * 128],
                             d[:, 3, k, :]),
                        )
                        for ci_, (src, dd) in enumerate(corners):
                            mm = tensor.matmul(ps_t[jj % NPST][:, q, :], src, dd,
                                               start=(ci_ == 0), stop=(ci_ == 3))
                        if q == 3 or t == N_CKT - 1:
                            mm.then_inc(s_tr, 1)
                        j += 1
                if c % 4 == 3:
                    g = c // 4
                    tensor.wait_ge(s_cpy, 5 * (c + 1))
                    for ob in range(2):
                        if "pe" in skip:
                            tensor.sem_inc(s_mm, 1)
                            continue
                        if g >= 1:
                            tensor.wait_ge(s_oc, 2 * (g - 1) + ob + 1)
                        for t in range(N_CKT):
                            mm = tensor.matmul(
                                ps_g[ob][:],
                                wt_sb[:, t, ob * 128:(ob + 1) * 128],
                                cols[:, t, g * GEMM_NP:(g + 1) * GEMM_NP],
                                start=(t == 0), stop=(t == N_CKT - 1))
                        mm.then_inc(s_mm, 1)

        @block.scalar
        def _(scalar):
            j = 0
            scalar.wait_ge(s_wts, 1)
            for c in range(N_PB):
                # diag builds for corners BL, BR of chunk c
                if c >= 2:
                    scalar.wait_ge(s_tr, 5 * (c - 1))
                if "combine" in skip:
                    scalar.sem_inc(s_dga, 1)
                else:
                    d = dbuf[c % 2]
                    for k in range(KK):
                        mmm = scalar.activation(
                            d[:, 3, k, :], id_sb[:], AF.Copy,
                            scale=wBR[:, KK * c + k:KK * c + k + 1])
                    mmm.then_inc(s_dga, 1)
                for g4 in range(5):
                    nt = 4 if g4 < 4 else 2      # tiles in this group (18 = 4*4+2)
                    jj = c * 5 + g4
                    if "act" in skip:
                        scalar.sem_inc(s_cpy, 1)
                        continue
                    scalar.wait_ge(s_tr, jj + 1)
                    t0_ = g4 * 4
                    scalar.activation(cols[:, t0_:t0_ + nt, c * 128:(c + 1) * 128],
                                      ps_t[jj % NPST][:, 0:nt, :],
                                      AF.Copy).then_inc(s_cpy, 1)
                if c % 4 == 3:
                    g = c // 4
                    for ob in range(2):
                        jj = 2 * g + ob
                        if "act" in skip:
                            scalar.sem_inc(s_oc, 1)
                            continue
                        scalar.wait_ge(s_mm, jj + 1)
                        if g >= 1:
                            scalar.wait_ge(s_st[ob], 16 * g)
                        scalar.activation(out_sb[ob][:], ps_g[ob][:],
                                          AF.Copy).then_inc(s_oc, 1)

        blk_cm.__exit__(None, None, None)

    snap = nc._state.snapshot_sems()
    for rep in range(reps):
        emit_body()
        if rep < reps - 1:
            nc.clear_and_free_semaphores(nc._state.allocated_since(snap))
            nc.all_engine_barrier()
            nc._state.restore_sems(snap)

    st.close()
    nc.compile()
    return nc


def _host_prep(x, offset, weight):
    """Build the 8 per-core input maps."""
    f16 = np.float16
    # xT fp16, padded with one zero row: [HW+1, C]
    xts = []
    for b in range(B):
        xt = np.zeros(((HW + 1) * C,), dtype=f16)
        xt[:HW * C] = np.ascontiguousarray(
            x[b].reshape(C, HW).T).astype(f16).reshape(-1)
        xts.append(xt)
    # weights: wt[t, c, o] = weight[o, cb*128+c, ky, kx],  t = 2*(3*ky+kx) + cb
    wr = weight.reshape(O, C, KK).transpose(2, 1, 0)   # [KK, C, O]
    wt = np.empty((N_CKT, 128, O), dtype=f16)
    for k in range(KK):
        for cb in range(2):
            wt[2 * k + cb] = wr[k, cb * 128:(cb + 1) * 128, :].astype(f16)

    ident = np.eye(128, dtype=f16)

    ky, kx = np.meshgrid(np.arange(3), np.arange(3), indexing="ij")
    ky = ky.reshape(-1).astype(np.float32)   # [KK]
    kx = kx.reshape(-1).astype(np.float32)

    in_maps = []
    for core in range(N_CORES):
        b, hhalf = core // 2, core % 2
        i0 = hhalf * ROWS_PER_CORE
        off = offset[b].reshape(KK, 2, H, W)[:, :, i0:i0 + ROWS_PER_CORE, :]
        offy = off[:, 0].reshape(KK, P_CORE).astype(np.float32)   # [KK, P]
        offx = off[:, 1].reshape(KK, P_CORE).astype(np.float32)
        p = np.arange(P_CORE)
        basy = (i0 + p // W - 1).astype(np.float32)[None, :] + ky[:, None]  # [KK, P]
        basx = (p % W - 1).astype(np.float32)[None, :] + kx[:, None]

        # w-layout [128, N_SLOT]: (part, KK*pb + k) = val[k, pb*128 + part]
        def to_w(a):
            # a: [KK, P_CORE] -> [128, N_PB, KK] -> [128, N_SLOT]
            return np.ascontiguousarray(
                a.reshape(KK, N_PB, 128).transpose(2, 1, 0)).reshape(128, N_SLOT)

        # g-layout [128, NG]: (16g + l, 72*pb + 8*k + w) = val[k, pb*128 + w*16 + l]
        def to_g(a):
            a4 = a.reshape(KK, N_PB, 8, 16)          # [k, pb, w, l]
            g1 = a4.transpose(3, 1, 0, 2)            # [l, pb, k, w]
            g1 = np.ascontiguousarray(g1).reshape(16, N_IDX // 16)
            return np.tile(g1, (8, 1))               # replicate to 128 partitions

        in_maps.append({
            "xt": xts[b], "wt": wt, "ident": ident,
            "offy_w": to_w(offy), "offx_w": to_w(offx),
            "basy_w": to_w(np.broadcast_to(basy, offy.shape)),
            "basx_w": to_w(np.broadcast_to(basx, offx.shape)),
            "offy_g": to_g(offy), "offx_g": to_g(offx),
            "basy_g": to_g(np.broadcast_to(basy, offy.shape)),
            "basx_g": to_g(np.broadcast_to(basx, offx.shape)),
        })
    return in_maps


def _assemble(results):
    out = np.empty((B, O, H, W), dtype=np.float32)
    for core in range(N_CORES):
        b, hhalf = core // 2, core % 2
        i0 = hhalf * ROWS_PER_CORE
        out[b, :, i0:i0 + ROWS_PER_CORE, :] = \
            np.asarray(results[core]["out"]).reshape(O, ROWS_PER_CORE, W)
    return out


def _make_exec(nc, donate=False):
    """Build a cached jitted SPMD executor for a compiled Bass module.

    Replicates concourse.bass2jax.run_bass_via_pjrt's lowering (same
    _bass_exec_p bind / shard_map layout) but returns a reusable jitted
    callable, so repeated invocations skip re-trace/re-lower/re-compile.
    """
    import jax
    import numpy as _np
    from jax.sharding import Mesh, PartitionSpec
    from jax.experimental.shard_map import shard_map
    from concourse import bass2jax
    import concourse.mybir as mybir

    bass2jax.install_neuronx_cc_hook()
    assert nc.dbg_addr is None
    partition_name = (nc.partition_id_tensor.name
                      if nc.partition_id_tensor else None)

    in_names, out_names, out_avals, zero_outs = [], [], [], []
    for alloc in nc.m.functions[0].allocations:
        if not isinstance(alloc, mybir.MemoryLocationSet):
            continue
        name = alloc.memorylocations[0].name
        if alloc.kind == "ExternalInput":
            if name != partition_name:
                in_names.append(name)
        elif alloc.kind == "ExternalOutput":
            out_names.append(name)
            shape = tuple(alloc.tensor_shape)
            dtype = mybir.dt.np(alloc.dtype)
            out_avals.append(jax.core.ShapedArray(shape, dtype))
            zero_outs.append(_np.zeros(shape, dtype))
    n_params = len(in_names)
    all_names = list(in_names) + list(out_names)
    if partition_name is not None:
        all_names.append(partition_name)
    all_names = tuple(all_names)

    def _body(*args):
        operands = list(args)
        if partition_name is not None:
            operands.append(bass2jax.partition_id_tensor())
        outs = bass2jax._bass_exec_p.bind(
            *operands,
            out_avals=tuple(out_avals),
            in_names=all_names,
            out_names=tuple(out_names),
            lowering_input_output_aliases=(),
            sim_require_finite=True,
            sim_require_nnan=True,
            nc=nc,
        )
        return tuple(outs)

    devices = jax.devices()[:N_CORES]
    mesh = Mesh(np.asarray(devices), ("core",))
    n_out = len(out_names)
    fn = jax.jit(
        shard_map(_body, mesh=mesh,
                  in_specs=(PartitionSpec("core"),) * (n_params + n_out),
                  out_specs=(PartitionSpec("core"),) * n_out,
                  check_rep=False),
        donate_argnums=tuple(range(n_params, n_params + n_out)) if donate else (),
        keep_unused=True,
    )
    return {"fn": fn, "in_names": in_names, "out_names": out_names,
            "zero_outs": zero_outs, "mesh": mesh, "n_params": n_params}


def _concat_inputs(ex, in_maps):
    return [np.concatenate([in_maps[c][n] for c in range(N_CORES)], axis=0)
            for n in ex["in_names"]]


def _concat_zeros(ex):
    return [np.zeros((N_CORES * z.shape[0], *z.shape[1:]), z.dtype)
            for z in ex["zero_outs"]]


def kernel(x, offset, weight):
    import jax
    x = np.asarray(x, dtype=np.float32)
    offset = np.asarray(offset, dtype=np.float32)
    weight = np.asarray(weight, dtype=np.float32)
    if "nc" not in _CACHE:
        _CACHE["nc"] = _build_nc()
    if "exec" not in _CACHE:
        _CACHE["exec"] = _make_exec(_CACHE["nc"])
    ex = _CACHE["exec"]
    in_maps = _host_prep(x, offset, weight)
    outs = ex["fn"](*_concat_inputs(ex, in_maps), *_concat_zeros(ex))
    full = np.asarray(outs[0]).reshape(N_CORES, O, P_CORE)
    results = [{"out": full[c]} for c in range(N_CORES)]
    return _assemble(results)



# revision 24
# speedup vs baseline: 1.0230x; 1.0230x over previous
"""Deformable conv (B=4, C=256, H=W=64, O=256, K=3, pad=1) on 8 NeuronCores.

Sharding: core = (image b, row-half h): each core computes out[b, :, h*32:(h+1)*32, :].

Per-core device pipeline:
  1. DVE computes gather indices (int16, wrapped-16 layout) and bilinear corner
     weights (f32, per-(pos,tap) scalars) from the raw conv offsets.
  2. GpSimd dma_gathers fp16 channel-pair rows (512ch covering x0,x0+1) from a
     host-transposed xT [HW, C] fp16 image in DRAM: 2 gathers (top/bottom row)
     per (tap, position); 1KB descriptors; NBUF=3-deep buffer rotation.
  3. DVE (corners TL/TR/BL) and Act (corner BR) expand the per-(pos,tap)
     corner weights into dense diagonal matrices d = id * w.
  4. PE fuses transpose + 4-corner bilinear combine: per (tap, ch-block) tile,
     4 accumulating fp16 matmuls g_tile[pos, ch] @ diag(w_corner) land the
     weighted [ch, pos] tile in fp32 PSUM; Act copies tiles into the fp16
     cols buffer; PE then runs the O=256 x CK=2304 x P=2048 GEMM with fp32
     PSUM accumulation.

Execution: _make_exec builds a cached jitted shard_map executor (same
_bass_exec_p lowering as bass2jax.run_bass_via_pjrt) so repeated calls skip
re-trace/re-lower/re-compile.
"""

import numpy as np

B, C, H, W = 4, 256, 64, 64
O, KK = 256, 9
HW = H * W                      # 4096
ROWS_PER_CORE = H // 2          # 32
P_CORE = ROWS_PER_CORE * W      # 2048 output positions per core
N_PB = P_CORE // 128            # 16 chunks (position blocks)
IDX_PER_CHUNK = KK * 128        # 1152
N_IDX = N_PB * IDX_PER_CHUNK    # 18432
N_SLOT = N_PB * KK              # 144
N_CKT = 2 * KK                  # 18 ck-tiles of 128
GEMM_NP = 512                   # positions per GEMM pass
N_GP = P_CORE // GEMM_NP        # 4 GEMM passes
N_CORES = 8

_CACHE = {}


def _build_nc(reps=1, skip=()):
    skip = set(skip)
    import concourse.bacc as bacc
    import concourse.mybir as mybir
    from concourse import library_config
    import bass_rust

    F32, F16, I16, I32 = (mybir.dt.float32, mybir.dt.float16,
                          mybir.dt.int16, mybir.dt.int32)
    AF = mybir.ActivationFunctionType
    AL = mybir.AluOpType

    nc = bacc.Bacc("TRN2")

    # ---- DRAM parameters (per-core inputs) ----
    xt = nc.declare_dram_parameter("xt", [(HW + 1) * C], F16, isOutput=False)
    wt = nc.declare_dram_parameter("wt", [N_CKT, 128, O], F16, isOutput=False)
    offy_w = nc.declare_dram_parameter("offy_w", [128, N_SLOT], F32, isOutput=False)
    offx_w = nc.declare_dram_parameter("offx_w", [128, N_SLOT], F32, isOutput=False)
    basy_w = nc.declare_dram_parameter("basy_w", [128, N_SLOT], F32, isOutput=False)
    basx_w = nc.declare_dram_parameter("basx_w", [128, N_SLOT], F32, isOutput=False)
    offy_g = nc.declare_dram_parameter("offy_g", [128, N_IDX // 16], F32, isOutput=False)
    offx_g = nc.declare_dram_parameter("offx_g", [128, N_IDX // 16], F32, isOutput=False)
    basy_g = nc.declare_dram_parameter("basy_g", [128, N_IDX // 16], F32, isOutput=False)
    basx_g = nc.declare_dram_parameter("basx_g", [128, N_IDX // 16], F32, isOutput=False)
    ident = nc.declare_dram_parameter("ident", [128, 128], F16, isOutput=False)
    out = nc.declare_dram_parameter("out", [O, P_CORE], F32, isOutput=True)

    NG = N_IDX // 16     # 1152 free-dim of g-layout

    from contextlib import ExitStack
    st = ExitStack()
    sb = lambda n, s, d: st.enter_context(nc.sbuf_tensor(n, s, d))
    ps = lambda n, s, d: st.enter_context(nc.psum_tensor(n, s, d))

    # ---- SBUF ----
    idx_top = sb("idx_top", [128, NG], I16)
    idx_bot = sb("idx_bot", [128, NG], I16)
    # w-layout pipeline
    ow_y = sb("ow_y", [128, N_SLOT], F32)
    ow_x = sb("ow_x", [128, N_SLOT], F32)
    bw_y = sb("bw_y", [128, N_SLOT], F32)
    bw_x = sb("bw_x", [128, N_SLOT], F32)
    u0 = sb("u0", [128, N_SLOT], F32)
    u1 = sb("u1", [128, N_SLOT], F32)
    u2 = sb("u2", [128, N_SLOT], F32)
    ui = sb("ui", [128, N_SLOT], I32)
    fy = sb("fy", [128, N_SLOT], F32)
    fx = sb("fx", [128, N_SLOT], F32)
    vy0 = sb("vy0", [128, N_SLOT], F32)
    vy1 = sb("vy1", [128, N_SLOT], F32)
    vx0 = sb("vx0", [128, N_SLOT], F32)
    vx1 = sb("vx1", [128, N_SLOT], F32)
    sel = sb("sel", [128, N_SLOT], F32)
    qy0 = sb("qy0", [128, N_SLOT], F32)
    qy1 = sb("qy1", [128, N_SLOT], F32)
    wlx = sb("wlx", [128, N_SLOT], F32)
    wrx = sb("wrx", [128, N_SLOT], F32)
    wTL = sb("wTL", [128, N_SLOT], F32)
    wTR = sb("wTR", [128, N_SLOT], F32)
    wBL = sb("wBL", [128, N_SLOT], F32)
    wBR = sb("wBR", [128, N_SLOT], F32)
    # transposed cols, full [128, N_CKT, P_CORE] fp16
    cols = sb("cols", [128, N_CKT, P_CORE], F16)
    # weights in SBUF [128, N_CKT, O] fp16
    wt_sb = sb("wt_sb", [128, N_CKT, O], F16)
    id_sb = sb("id_sb", [128, 128], F16)
    out_sb = [sb(f"out_sb{i}", [128, GEMM_NP], F32) for i in range(2)]

    # g-layout coord pipeline: temps are dead once idx_top/idx_bot are
    # written, so they live in a nested stack (top of the SBUF allocation
    # stack) that is closed before the gather buffers are allocated — the
    # allocator reuses the space. Runtime-safe: the first gather write is
    # gated on s_idx, which fires after the last coord-temp read.
    st_coord = ExitStack()
    sbc = lambda n, s, d: st_coord.enter_context(nc.sbuf_tensor(n, s, d))
    og_y = sbc("og_y", [128, NG], F32)
    og_x = sbc("og_x", [128, NG], F32)
    bg_y = sbc("bg_y", [128, NG], F32)
    bg_x = sbc("bg_x", [128, NG], F32)
    t0 = sbc("t0", [128, NG], F32)
    t1 = sbc("t1", [128, NG], F32)
    t2 = sbc("t2", [128, NG], F32)
    ti = sbc("ti", [128, NG], I32)
    s_clip = sbc("s_clip", [128, NG], F32)
    st_coord.close()

    # gather buffers (NBUF-deep): [128, KK, 512] fp16 — reuse coord space
    NBUF = 3
    g_top = [sb(f"g_top{i}", [128, KK, 512], F16) for i in range(NBUF)]
    g_bot = [sb(f"g_bot{i}", [128, KK, 512], F16) for i in range(NBUF)]
    # per-(corner, tap) diagonal weight matrices, double buffered:
    # dbuf[b][:, corner, k, :] = diag(w_corner[:, KK*c+k]) as dense [128,128]
    dbuf = [sb(f"dbuf{i}", [128, 4, KK, 128], F16) for i in range(2)]

    NPST = 3
    ps_t = [ps(f"ps_t{i}", [128, 4, 128], F32) for i in range(NPST)]
    ps_g = [ps(f"ps_g{i}", [128, GEMM_NP], F32) for i in range(2)]

    N_LOADS = 10
    IO_ALL = N_LOADS * 16

    def emit_body():
        s_io = nc.alloc_semaphore("s_io")
        s_idx = nc.alloc_semaphore("s_idx")
        s_wts = nc.alloc_semaphore("s_wts")
        s_g = [nc.alloc_semaphore(f"s_g{i}") for i in range(2 * NBUF)]
        s_dg = nc.alloc_semaphore("s_dg")
        s_dga = nc.alloc_semaphore("s_dga")
        s_tr = nc.alloc_semaphore("s_tr")
        s_cpy = nc.alloc_semaphore("s_cpy")
        s_mm = nc.alloc_semaphore("s_mm")
        s_oc = nc.alloc_semaphore("s_oc")
        s_st = [nc.alloc_semaphore(f"s_st{i}") for i in range(2)]
        blk_cm = nc.Block()
        block = blk_cm.__enter__()

        @block.sync
        def _(sync):
            sync.dma_start(wt_sb[:], wt.rearrange("t c e -> c t e")).then_inc(s_io, 16)
            for src, dst in [(offy_w, ow_y), (offx_w, ow_x), (basy_w, bw_y),
                             (basx_w, bw_x), (offy_g, og_y), (offx_g, og_x),
                             (basy_g, bg_y), (basx_g, bg_x), (ident, id_sb)]:
                sync.dma_start(dst[:], src[:]).then_inc(s_io, 16)
            for g in range(N_GP):
                for ob in range(2):
                    j = 2 * g + ob
                    sync.wait_ge(s_oc, j + 1)
                    sync.dma_start(out[ob * 128:(ob + 1) * 128,
                                       g * GEMM_NP:(g + 1) * GEMM_NP],
                                   out_sb[ob][:]).then_inc(s_st[ob], 16)
            sync.wait_ge(s_st[0], 16 * N_GP)
            sync.wait_ge(s_st[1], 16 * N_GP)

        @block.vector
        def _(vector):
            vector.wait_ge(s_io, IO_ALL)

            def TT(out_, a, b, op):
                vector.tensor_tensor(out_, a, b, op)
                vector.drain()

            def TS(out_, a, s1, s2, op0, op1=None):
                if op1 is None:
                    vector.tensor_scalar(out_, a, s1, None, op0)
                else:
                    vector.tensor_scalar(out_, a, s1, s2, op0, op1)
                vector.drain()

            def CP(out_, a):
                vector.tensor_copy(out_, a)
                vector.drain()

            def STT(out_, a, sc, b, op0, op1):
                vector.scalar_tensor_tensor(out_, a, sc, b, op0, op1)
                vector.drain()

            def floor_to(dst_f, dst_frac, src, tmp_i, tmp_a, tmp_b):
                # dst_f = floor(src); dst_frac = src - floor(src)
                CP(tmp_i[:], src[:])                 # rint cast
                CP(tmp_a[:], tmp_i[:])               # back to f32
                TT(tmp_b[:], tmp_a[:], src[:], AL.is_gt)
                TT(dst_f[:], tmp_a[:], tmp_b[:], AL.subtract)
                if dst_frac is not None:
                    TT(dst_frac[:], src[:], dst_f[:], AL.subtract)

            # ---- idx computation (g layout) ----
            TT(t0[:], og_y[:], bg_y[:], AL.add)
            TS(t0[:], t0[:], -8.0, 72.0, AL.max, AL.min)
            floor_to(t1, None, t0, ti, t2, s_clip)
            TT(t0[:], og_x[:], bg_x[:], AL.add)
            TS(t0[:], t0[:], -8.0, 72.0, AL.max, AL.min)
            floor_to(t2, None, t0, ti, s_clip, og_x)   # og_x clobbered as tmp
            TS(s_clip[:], t2[:], 0.0, 63.0, AL.max, AL.min)
            TS(t0[:], t1[:], 0.0, 63.0, AL.max, AL.min)
            STT(t0[:], t0[:], 64.0, s_clip[:], AL.mult, AL.add)
            CP(idx_top[:], t0[:])
            TS(t0[:], t1[:], 1.0, 0.0, AL.add, AL.max)
            TS(t0[:], t0[:], 63.0, None, AL.min)
            STT(t0[:], t0[:], 64.0, s_clip[:], AL.mult, AL.add)
            vector.tensor_copy(idx_bot[:], t0[:])
            vector.drain().then_inc(s_idx, 1)

            # ---- weight computation (w layout) ----
            TT(u0[:], ow_y[:], bw_y[:], AL.add)
            TS(u0[:], u0[:], -8.0, 72.0, AL.max, AL.min)
            floor_to(u1, fy, u0, ui, u2, vy0)
            TS(vy0[:], u1[:], 0.0, None, AL.is_ge)
            TS(u2[:], u1[:], 63.0, None, AL.is_le)
            TT(vy0[:], vy0[:], u2[:], AL.mult)
            TS(vy1[:], u1[:], -1.0, None, AL.is_ge)
            TS(u2[:], u1[:], 62.0, None, AL.is_le)
            TT(vy1[:], vy1[:], u2[:], AL.mult)

            TT(u0[:], ow_x[:], bw_x[:], AL.add)
            TS(u0[:], u0[:], -8.0, 72.0, AL.max, AL.min)
            floor_to(u1, fx, u0, ui, u2, vx0)
            TS(vx0[:], u1[:], 0.0, None, AL.is_ge)
            TS(u2[:], u1[:], 63.0, None, AL.is_le)
            TT(vx0[:], vx0[:], u2[:], AL.mult)
            TS(vx1[:], u1[:], -1.0, None, AL.is_ge)
            TS(u2[:], u1[:], 62.0, None, AL.is_le)
            TT(vx1[:], vx1[:], u2[:], AL.mult)
            TS(sel[:], u1[:], -1.0, None, AL.is_equal)

            TS(u0[:], fy[:], -1.0, 1.0, AL.mult, AL.add)
            TT(qy0[:], u0[:], vy0[:], AL.mult)
            TT(qy1[:], fy[:], vy1[:], AL.mult)
            TS(u0[:], fx[:], -1.0, 1.0, AL.mult, AL.add)
            TT(u0[:], u0[:], vx0[:], AL.mult)
            TT(u1[:], fx[:], vx1[:], AL.mult)
            TT(u2[:], u1[:], u0[:], AL.subtract)
            TT(u2[:], u2[:], sel[:], AL.mult)
            TT(wlx[:], u0[:], u2[:], AL.add)
            TT(u2[:], u1[:], sel[:], AL.mult)
            TT(wrx[:], u1[:], u2[:], AL.subtract)
            TT(wTL[:], qy0[:], wlx[:], AL.mult)
            TT(wTR[:], qy0[:], wrx[:], AL.mult)
            TT(wBL[:], qy1[:], wlx[:], AL.mult)
            vector.tensor_tensor(wBR[:], qy1[:], wrx[:], AL.mult)
            vector.drain().then_inc(s_wts, 1)

            # ---- diag weight builds (corners TL, TR; Act builds BL, BR) ----
            # dbuf[c%2][:, j, k, :] = diag(w_j[:, KK*c+k]) = id * w (scalar
            # per partition); the PE then applies the corner combine as
            # 4 accumulating matmuls g_tile @ diag per (k, cb) tile.
            for c in range(N_PB):
                if c >= 2:
                    vector.wait_ge(s_tr, 5 * (c - 1))
                d = dbuf[c % 2]
                if "combine" in skip:
                    vector.drain().then_inc(s_dg, 1)
                    continue
                for j2, w_ in ((0, wTL), (1, wTR), (2, wBL)):
                    for k in range(KK):
                        vector.tensor_scalar(d[:, j2, k, :], id_sb[:],
                                             w_[:, KK * c + k:KK * c + k + 1],
                                             None, AL.mult)
                vector.drain().then_inc(s_dg, 1)

        @block.gpsimd
        def _(gpsimd):
            gpsimd.load_library(library_config.mlp)
            gpsimd.wait_ge(s_idx, 1)
            xt_view = xt[:].copy()
            xt_view.ap = bass_rust.VecI64Pair([[C, HW], [1, 2 * C]])
            for c in range(N_PB):
                if c >= NBUF:
                    gpsimd.wait_ge(s_tr, 5 * (c - NBUF + 1))
                i0 = c * 72
                # one dma_gather tops out at 1024 idx -> split 1152 = 1024 + 128
                for (lo, hi, s0, s1) in ((0, 64, 0, 8), (64, 72, 8, 9)):
                    nidx = (hi - lo) * 16
                    if "gather" in skip:
                        gpsimd.sem_inc(s_g[c % NBUF], 16)
                        gpsimd.sem_inc(s_g[NBUF + c % NBUF], 16)
                        continue
                    gpsimd.dma_gather(g_top[c % NBUF][:, s0:s1, :], xt_view,
                                      idx_top[:, i0 + lo:i0 + hi],
                                      nidx, nidx, 2 * C,
                                      elem_step=C).then_inc(s_g[c % NBUF], 16)
                    gpsimd.dma_gather(g_bot[c % NBUF][:, s0:s1, :], xt_view,
                                      idx_bot[:, i0 + lo:i0 + hi],
                                      nidx, nidx, 2 * C,
                                      elem_step=C).then_inc(s_g[NBUF + c % NBUF], 16)

        @block.tensor
        def _(tensor):
            tensor.wait_ge(s_io, IO_ALL)
            j = 0
            for c in range(N_PB):
                tensor.wait_ge(s_dg, c + 1)
                tensor.wait_ge(s_dga, c + 1)
                tensor.wait_ge(s_g[c % NBUF], 32 * (c // NBUF + 1))
                tensor.wait_ge(s_g[NBUF + c % NBUF], 32 * (c // NBUF + 1))
                gt, gb = g_top[c % NBUF], g_bot[c % NBUF]
                d = dbuf[c % 2]
                for k in range(KK):
                    for cb in range(2):
                        t = 2 * k + cb
                        g4, q = t // 4, t % 4
                        jj = c * 5 + g4          # global 4-group index (5 groups/chunk)
                        if "pe" in skip:
                            if q == 0:
                                tensor.sem_inc(s_tr, 1)
                            j += 1
                            continue
                        if q == 0 and jj >= NPST:
                            tensor.wait_ge(s_cpy, jj - NPST + 1)
                        corners = (
                            (gt[:, k, cb * 128:(cb + 1) * 128], d[:, 0, k, :]),
                            (gt[:, k, 256 + cb * 128:256 + (cb + 1) * 128],
                             d[:, 1, k, :]),
                            (gb[:, k, cb * 128:(cb + 1) * 128], d[:, 2, k, :]),
                            (gb[:, k, 256 + cb * 128:256 + (cb + 1) * 128],
                             d[:, 3, k, :]),
                        )
                        for ci_, (src, dd) in enumerate(corners):
                            mm = tensor.matmul(ps_t[jj % NPST][:, q, :], src, dd,
                                               start=(ci_ == 0), stop=(ci_ == 3))
                        if q == 3 or t == N_CKT - 1:
                            mm.then_inc(s_tr, 1)
                        j += 1
                if c % 4 == 3:
                    g = c // 4
                    tensor.wait_ge(s_cpy, 5 * (c + 1))
                    for ob in range(2):
                        if "pe" in skip:
                            tensor.sem_inc(s_mm, 1)
                            continue
                        if g >= 1:
                            tensor.wait_ge(s_oc, 2 * (g - 1) + ob + 1)
                        for t in range(N_CKT):
                            mm = tensor.matmul(
                                ps_g[ob][:],
                                wt_sb[:, t, ob * 128:(ob + 1) * 128],
                                cols[:, t, g * GEMM_NP:(g + 1) * GEMM_NP],
                                start=(t == 0), stop=(t == N_CKT - 1))
                        mm.then_inc(s_mm, 1)

        @block.scalar
        def _(scalar):
            j = 0
            scalar.wait_ge(s_wts, 1)
            for c in range(N_PB):
                # diag builds for corners BL, BR of chunk c
                if c >= 2:
                    scalar.wait_ge(s_tr, 5 * (c - 1))
                if "combine" in skip:
                    scalar.sem_inc(s_dga, 1)
                else:
                    d = dbuf[c % 2]
                    for k in range(KK):
                        mmm = scalar.activation(
                            d[:, 3, k, :], id_sb[:], AF.Copy,
                            scale=wBR[:, KK * c + k:KK * c + k + 1])
                    mmm.then_inc(s_dga, 1)
                for g4 in range(5):
                    nt = 4 if g4 < 4 else 2      # tiles in this group (18 = 4*4+2)
                    jj = c * 5 + g4
                    if "act" in skip:
                        scalar.sem_inc(s_cpy, 1)
                        continue
                    scalar.wait_ge(s_tr, jj + 1)
                    t0_ = g4 * 4
                    scalar.activation(cols[:, t0_:t0_ + nt, c * 128:(c + 1) * 128],
                                      ps_t[jj % NPST][:, 0:nt, :],
                                      AF.Copy).then_inc(s_cpy, 1)
                if c % 4 == 3:
                    g = c // 4
                    for ob in range(2):
                        jj = 2 * g + ob
                        if "act" in skip:
                            scalar.sem_inc(s_oc, 1)
                            continue
                        scalar.wait_ge(s_mm, jj + 1)
                        if g >= 1:
                            scalar.wait_ge(s_st[ob], 16 * g)
                        scalar.activation(out_sb[ob][:], ps_g[ob][:],
                                          AF.Copy).then_inc(s_oc, 1)

        blk_cm.__exit__(None, None, None)

    snap = nc._state.snapshot_sems()
    for rep in range(reps):
        emit_body()
        if rep < reps - 1:
            nc.clear_and_free_semaphores(nc._state.allocated_since(snap))
            nc.all_engine_barrier()
            nc._state.restore_sems(snap)

    st.close()
    nc.compile()
    return nc


def _host_prep(x, offset, weight):
    """Build the 8 per-core input maps."""
    f16 = np.float16
    # xT fp16, padded with one zero row: [HW+1, C]
    xts = []
    for b in range(B):
        xt = np.zeros(((HW + 1) * C,), dtype=f16)
        xt[:HW * C] = np.ascontiguousarray(
            x[b].reshape(C, HW).T).astype(f16).reshape(-1)
        xts.append(xt)
    # weights: wt[t, c, o] = weight[o, cb*128+c, ky, kx],  t = 2*(3*ky+kx) + cb
    wr = weight.reshape(O, C, KK).transpose(2, 1, 0)   # [KK, C, O]
    wt = np.empty((N_CKT, 128, O), dtype=f16)
    for k in range(KK):
        for cb in range(2):
            wt[2 * k + cb] = wr[k, cb * 128:(cb + 1) * 128, :].astype(f16)

    ident = np.eye(128, dtype=f16)

    ky, kx = np.meshgrid(np.arange(3), np.arange(3), indexing="ij")
    ky = ky.reshape(-1).astype(np.float32)   # [KK]
    kx = kx.reshape(-1).astype(np.float32)

    in_maps = []
    for core in range(N_CORES):
        b, hhalf = core // 2, core % 2
        i0 = hhalf * ROWS_PER_CORE
        off = offset[b].reshape(KK, 2, H, W)[:, :, i0:i0 + ROWS_PER_CORE, :]
        offy = off[:, 0].reshape(KK, P_CORE).astype(np.float32)   # [KK, P]
        offx = off[:, 1].reshape(KK, P_CORE).astype(np.float32)
        p = np.arange(P_CORE)
        basy = (i0 + p // W - 1).astype(np.float32)[None, :] + ky[:, None]  # [KK, P]
        basx = (p % W - 1).astype(np.float32)[None, :] + kx[:, None]

        # w-layout [128, N_SLOT]: (part, KK*pb + k) = val[k, pb*128 + part]
        def to_w(a):
            # a: [KK, P_CORE] -> [128, N_PB, KK] -> [128, N_SLOT]
            return np.ascontiguousarray(
                a.reshape(KK, N_PB, 128).transpose(2, 1, 0)).reshape(128, N_SLOT)

        # g-layout [128, NG]: (16g + l, 72*pb + 8*k + w) = val[k, pb*128 + w*16 + l]
        def to_g(a):
            a4 = a.reshape(KK, N_PB, 8, 16)          # [k, pb, w, l]
            g1 = a4.transpose(3, 1, 0, 2)            # [l, pb, k, w]
            g1 = np.ascontiguousarray(g1).reshape(16, N_IDX // 16)
            return np.tile(g1, (8, 1))               # replicate to 128 partitions

        in_maps.append({
            "xt": xts[b], "wt": wt, "ident": ident,
            "offy_w": to_w(offy), "offx_w": to_w(offx),
            "basy_w": to_w(np.broadcast_to(basy, offy.shape)),
            "basx_w": to_w(np.broadcast_to(basx, offx.shape)),
            "offy_g": to_g(offy), "offx_g": to_g(offx),
            "basy_g": to_g(np.broadcast_to(basy, offy.shape)),
            "basx_g": to_g(np.broadcast_to(basx, offx.shape)),
        })
    return in_maps


def _assemble(results):
    out = np.empty((B, O, H, W), dtype=np.float32)
    for core in range(N_CORES):
        b, hhalf = core // 2, core % 2
        i0 = hhalf * ROWS_PER_CORE
        out[b, :, i0:i0 + ROWS_PER_CORE, :] = \
            np.asarray(results[core]["out"]).reshape(O, ROWS_PER_CORE, W)
    return out


def _make_exec(nc, donate=False):
    """Build a cached jitted SPMD executor for a compiled Bass module.

    Replicates concourse.bass2jax.run_bass_via_pjrt's lowering (same
    _bass_exec_p bind / shard_map layout) but returns a reusable jitted
    callable, so repeated invocations skip re-trace/re-lower/re-compile.
    """
    import jax
    import numpy as _np
    from jax.sharding import Mesh, PartitionSpec
    from jax.experimental.shard_map import shard_map
    from concourse import bass2jax
    import concourse.mybir as mybir

    bass2jax.install_neuronx_cc_hook()
    assert nc.dbg_addr is None
    partition_name = (nc.partition_id_tensor.name
                      if nc.partition_id_tensor else None)

    in_names, out_names, out_avals, zero_outs = [], [], [], []
    for alloc in nc.m.functions[0].allocations:
        if not isinstance(alloc, mybir.MemoryLocationSet):
            continue
        name = alloc.memorylocations[0].name
        if alloc.kind == "ExternalInput":
            if name != partition_name:
                in_names.append(name)
        elif alloc.kind == "ExternalOutput":
            out_names.append(name)
            shape = tuple(alloc.tensor_shape)
            dtype = mybir.dt.np(alloc.dtype)
            out_avals.append(jax.core.ShapedArray(shape, dtype))
            zero_outs.append(_np.zeros(shape, dtype))
    n_params = len(in_names)
    all_names = list(in_names) + list(out_names)
    if partition_name is not None:
        all_names.append(partition_name)
    all_names = tuple(all_names)

    def _body(*args):
        operands = list(args)
        if partition_name is not None:
            operands.append(bass2jax.partition_id_tensor())
        outs = bass2jax._bass_exec_p.bind(
            *operands,
            out_avals=tuple(out_avals),
            in_names=all_names,
            out_names=tuple(out_names),
            lowering_input_output_aliases=(),
            sim_require_finite=True,
            sim_require_nnan=True,
            nc=nc,
        )
        return tuple(outs)

    devices = jax.devices()[:N_CORES]
    mesh = Mesh(np.asarray(devices), ("core",))
    n_out = len(out_names)
    fn = jax.jit(
        shard_map(_body, mesh=mesh,
                  in_specs=(PartitionSpec("core"),) * (n_params + n_out),
                  out_specs=(PartitionSpec("core"),) * n_out,
                  check_rep=False),
        donate_argnums=tuple(range(n_params, n_params + n_out)) if donate else (),
        keep_unused=True,
    )
    return {"fn": fn, "in_names": in_names, "out_names": out_names,
            "zero_outs": zero_outs, "mesh": mesh, "n_params": n_params}


def _concat_inputs(ex, in_maps):
    return [np.concatenate([in_maps[c][n] for c in range(N_CORES)], axis=0)
            for n in ex["in_names"]]


def _concat_zeros(ex):
    return [np.zeros((N_CORES * z.shape[0], *z.shape[1:]), z.dtype)
            for z in ex["zero_outs"]]


def kernel(x, offset, weight):
    import jax
    x = np.asarray(x, dtype=np.float32)
    offset = np.asarray(offset, dtype=np.float32)
    weight = np.asarray(weight, dtype=np.float32)
    if "nc" not in _CACHE:
        _CACHE["nc"] = _build_nc()
    if "exec" not in _CACHE:
        _CACHE["exec"] = _make_exec(_CACHE["nc"])
    ex = _CACHE["exec"]
    in_maps = _host_prep(x, offset, weight)
    outs = ex["fn"](*_concat_inputs(ex, in_maps), *_concat_zeros(ex))
    full = np.asarray(outs[0]).reshape(N_CORES, O, P_CORE)
    results = [{"out": full[c]} for c in range(N_CORES)]
    return _assemble(results)



# revision 28
# speedup vs baseline: 1.1262x; 1.1009x over previous
"""Deformable conv (B=4, C=256, H=W=64, O=256, K=3, pad=1) on 8 NeuronCores.

Sharding: core = (image b, row-half h): each core computes out[b, :, h*32:(h+1)*32, :].

Per-core device pipeline:
  1. DVE computes gather indices (int16, wrapped-16 layout) and bilinear corner
     weights (f32, per-(pos,tap) scalars) from the raw conv offsets.
  2. GpSimd dma_gathers fp16 channel-pair rows (512ch covering x0,x0+1) from a
     host-transposed xT [HW, C] fp16 image in DRAM: 2 gathers (top/bottom row)
     per (tap, position); 1KB descriptors; NBUF=3-deep buffer rotation.
  3. DVE (corners TL/TR/BL) and Act (corner BR) expand the per-(pos,tap)
     corner weights into dense diagonal matrices d = id * w.
  4. PE fuses transpose + 4-corner bilinear combine: per (tap, ch-block) tile,
     4 accumulating fp16 matmuls g_tile[pos, ch] @ diag(w_corner) land the
     weighted [ch, pos] tile in fp32 PSUM; Act copies tiles into the fp16
     cols buffer; PE then runs the O=256 x CK=2304 x P=2048 GEMM with fp32
     PSUM accumulation.

Execution: _make_exec builds a cached jitted shard_map executor (same
_bass_exec_p lowering as bass2jax.run_bass_via_pjrt) so repeated calls skip
re-trace/re-lower/re-compile.
"""

import numpy as np

B, C, H, W = 4, 256, 64, 64
O, KK = 256, 9
HW = H * W                      # 4096
ROWS_PER_CORE = H // 2          # 32
P_CORE = ROWS_PER_CORE * W      # 2048 output positions per core
N_PB = P_CORE // 128            # 16 chunks (position blocks)
IDX_PER_CHUNK = KK * 128        # 1152
N_IDX = N_PB * IDX_PER_CHUNK    # 18432
N_SLOT = N_PB * KK              # 144
N_CKT = 2 * KK                  # 18 ck-tiles of 128
GEMM_NP = 512                   # positions per GEMM pass
N_GP = P_CORE // GEMM_NP        # 4 GEMM passes
N_CORES = 8

_CACHE = {}


def _build_nc(reps=1, skip=()):
    skip = set(skip)
    import concourse.bacc as bacc
    import concourse.mybir as mybir
    from concourse import library_config
    import bass_rust

    F32, F16, I16, I32 = (mybir.dt.float32, mybir.dt.float16,
                          mybir.dt.int16, mybir.dt.int32)
    AF = mybir.ActivationFunctionType
    AL = mybir.AluOpType

    nc = bacc.Bacc("TRN2")

    # ---- DRAM parameters (per-core inputs) ----
    xt = nc.declare_dram_parameter("xt", [(HW + 1) * C], F16, isOutput=False)
    wt = nc.declare_dram_parameter("wt", [N_CKT, 128, O], F16, isOutput=False)
    offy_w = nc.declare_dram_parameter("offy_w", [128, N_SLOT], F32, isOutput=False)
    offx_w = nc.declare_dram_parameter("offx_w", [128, N_SLOT], F32, isOutput=False)
    basy_w = nc.declare_dram_parameter("basy_w", [128, N_SLOT], F32, isOutput=False)
    basx_w = nc.declare_dram_parameter("basx_w", [128, N_SLOT], F32, isOutput=False)
    offy_g = nc.declare_dram_parameter("offy_g", [128, N_IDX // 16], F32, isOutput=False)
    offx_g = nc.declare_dram_parameter("offx_g", [128, N_IDX // 16], F32, isOutput=False)
    basy_g = nc.declare_dram_parameter("basy_g", [128, N_IDX // 16], F32, isOutput=False)
    basx_g = nc.declare_dram_parameter("basx_g", [128, N_IDX // 16], F32, isOutput=False)
    ident = nc.declare_dram_parameter("ident", [128, 128], F16, isOutput=False)
    out = nc.declare_dram_parameter("out", [O, P_CORE], F32, isOutput=True)

    NG = N_IDX // 16     # 1152 free-dim of g-layout

    from contextlib import ExitStack
    st = ExitStack()
    sb = lambda n, s, d: st.enter_context(nc.sbuf_tensor(n, s, d))
    ps = lambda n, s, d: st.enter_context(nc.psum_tensor(n, s, d))

    # ---- SBUF ----
    idx_top = sb("idx_top", [128, NG], I16)
    idx_bot = sb("idx_bot", [128, NG], I16)
    # w-layout pipeline
    ow_y = sb("ow_y", [128, N_SLOT], F32)
    ow_x = sb("ow_x", [128, N_SLOT], F32)
    bw_y = sb("bw_y", [128, N_SLOT], F32)
    bw_x = sb("bw_x", [128, N_SLOT], F32)
    u0 = sb("u0", [128, N_SLOT], F32)
    u1 = sb("u1", [128, N_SLOT], F32)
    u2 = sb("u2", [128, N_SLOT], F32)
    ui = sb("ui", [128, N_SLOT], I32)
    fy = sb("fy", [128, N_SLOT], F32)
    fx = sb("fx", [128, N_SLOT], F32)
    vy0 = sb("vy0", [128, N_SLOT], F32)
    vy1 = sb("vy1", [128, N_SLOT], F32)
    vx0 = sb("vx0", [128, N_SLOT], F32)
    vx1 = sb("vx1", [128, N_SLOT], F32)
    sel = sb("sel", [128, N_SLOT], F32)
    qy0 = sb("qy0", [128, N_SLOT], F32)
    qy1 = sb("qy1", [128, N_SLOT], F32)
    wlx = sb("wlx", [128, N_SLOT], F32)
    wrx = sb("wrx", [128, N_SLOT], F32)
    wTL = sb("wTL", [128, N_SLOT], F32)
    wTR = sb("wTR", [128, N_SLOT], F32)
    wBL = sb("wBL", [128, N_SLOT], F32)
    wBR = sb("wBR", [128, N_SLOT], F32)
    # transposed cols, full [128, N_CKT, P_CORE] fp16
    cols = sb("cols", [128, N_CKT, P_CORE], F16)
    # weights in SBUF [128, N_CKT, O] fp16
    wt_sb = sb("wt_sb", [128, N_CKT, O], F16)
    id_sb = sb("id_sb", [128, 128], F16)
    out_sb = [sb(f"out_sb{i}", [128, GEMM_NP], F32) for i in range(2)]

    # g-layout coord pipeline: temps are dead once idx_top/idx_bot are
    # written, so they live in a nested stack (top of the SBUF allocation
    # stack) that is closed before the gather buffers are allocated — the
    # allocator reuses the space. Runtime-safe: the first gather write is
    # gated on s_idx, which fires after the last coord-temp read.
    st_coord = ExitStack()
    sbc = lambda n, s, d: st_coord.enter_context(nc.sbuf_tensor(n, s, d))
    og_y = sbc("og_y", [128, NG], F32)
    og_x = sbc("og_x", [128, NG], F32)
    bg_y = sbc("bg_y", [128, NG], F32)
    bg_x = sbc("bg_x", [128, NG], F32)
    t0 = sbc("t0", [128, NG], F32)
    t1 = sbc("t1", [128, NG], F32)
    t2 = sbc("t2", [128, NG], F32)
    ti = sbc("ti", [128, NG], I32)
    s_clip = sbc("s_clip", [128, NG], F32)
    st_coord.close()

    # gather buffers (NBUF-deep): [128, KK, 512] fp16 — reuse coord space
    NBUF = 3
    g_top = [sb(f"g_top{i}", [128, KK, 512], F16) for i in range(NBUF)]
    g_bot = [sb(f"g_bot{i}", [128, KK, 512], F16) for i in range(NBUF)]
    # per-(corner, tap) diagonal weight matrices, double buffered:
    # dbuf[b][:, corner, k, :] = diag(w_corner[:, KK*c+k]) as dense [128,128]
    dbuf = [sb(f"dbuf{i}", [128, 4, KK, 128], F16) for i in range(2)]

    NPST = 3
    ps_t = [ps(f"ps_t{i}", [128, 4, 128], F32) for i in range(NPST)]
    ps_g = [ps(f"ps_g{i}", [128, GEMM_NP], F32) for i in range(2)]

    N_LOADS = 10
    IO_ALL = N_LOADS * 16

    def emit_body():
        s_io = nc.alloc_semaphore("s_io")
        s_idx = nc.alloc_semaphore("s_idx")
        s_wts = nc.alloc_semaphore("s_wts")
        s_g = [nc.alloc_semaphore(f"s_g{i}") for i in range(2 * NBUF)]
        s_dg = nc.alloc_semaphore("s_dg")
        s_tr = nc.alloc_semaphore("s_tr")
        s_cpy = nc.alloc_semaphore("s_cpy")
        s_mm = nc.alloc_semaphore("s_mm")
        s_oc = nc.alloc_semaphore("s_oc")
        s_st = [nc.alloc_semaphore(f"s_st{i}") for i in range(2)]
        blk_cm = nc.Block()
        block = blk_cm.__enter__()

        @block.sync
        def _(sync):
            sync.dma_start(wt_sb[:], wt.rearrange("t c e -> c t e")).then_inc(s_io, 16)
            for src, dst in [(offy_w, ow_y), (offx_w, ow_x), (basy_w, bw_y),
                             (basx_w, bw_x), (offy_g, og_y), (offx_g, og_x),
                             (basy_g, bg_y), (basx_g, bg_x), (ident, id_sb)]:
                sync.dma_start(dst[:], src[:]).then_inc(s_io, 16)
            for g in range(N_GP):
                for ob in range(2):
                    j = 2 * g + ob
                    sync.wait_ge(s_oc, j + 1)
                    sync.dma_start(out[ob * 128:(ob + 1) * 128,
                                       g * GEMM_NP:(g + 1) * GEMM_NP],
                                   out_sb[ob][:]).then_inc(s_st[ob], 16)
            sync.wait_ge(s_st[0], 16 * N_GP)
            sync.wait_ge(s_st[1], 16 * N_GP)

        @block.vector
        def _(vector):
            vector.wait_ge(s_io, IO_ALL)

            def TT(out_, a, b, op):
                vector.tensor_tensor(out_, a, b, op)
                vector.drain()

            def TS(out_, a, s1, s2, op0, op1=None):
                if op1 is None:
                    vector.tensor_scalar(out_, a, s1, None, op0)
                else:
                    vector.tensor_scalar(out_, a, s1, s2, op0, op1)
                vector.drain()

            def CP(out_, a):
                vector.tensor_copy(out_, a)
                vector.drain()

            def STT(out_, a, sc, b, op0, op1):
                vector.scalar_tensor_tensor(out_, a, sc, b, op0, op1)
                vector.drain()

            def floor_to(dst_f, dst_frac, src, tmp_i, tmp_a, tmp_b):
                # dst_f = floor(src); dst_frac = src - floor(src)
                CP(tmp_i[:], src[:])                 # rint cast
                CP(tmp_a[:], tmp_i[:])               # back to f32
                TT(tmp_b[:], tmp_a[:], src[:], AL.is_gt)
                TT(dst_f[:], tmp_a[:], tmp_b[:], AL.subtract)
                if dst_frac is not None:
                    TT(dst_frac[:], src[:], dst_f[:], AL.subtract)

            # ---- idx computation (g layout) ----
            TT(t0[:], og_y[:], bg_y[:], AL.add)
            TS(t0[:], t0[:], -8.0, 72.0, AL.max, AL.min)
            floor_to(t1, None, t0, ti, t2, s_clip)
            TT(t0[:], og_x[:], bg_x[:], AL.add)
            TS(t0[:], t0[:], -8.0, 72.0, AL.max, AL.min)
            floor_to(t2, None, t0, ti, s_clip, og_x)   # og_x clobbered as tmp
            TS(s_clip[:], t2[:], 0.0, 63.0, AL.max, AL.min)
            TS(t0[:], t1[:], 0.0, 63.0, AL.max, AL.min)
            STT(t0[:], t0[:], 64.0, s_clip[:], AL.mult, AL.add)
            CP(idx_top[:], t0[:])
            TS(t0[:], t1[:], 1.0, 0.0, AL.add, AL.max)
            TS(t0[:], t0[:], 63.0, None, AL.min)
            STT(t0[:], t0[:], 64.0, s_clip[:], AL.mult, AL.add)
            vector.tensor_copy(idx_bot[:], t0[:])
            vector.drain().then_inc(s_idx, 1)

            # ---- weight computation (w layout) ----
            TT(u0[:], ow_y[:], bw_y[:], AL.add)
            TS(u0[:], u0[:], -8.0, 72.0, AL.max, AL.min)
            floor_to(u1, fy, u0, ui, u2, vy0)
            TS(vy0[:], u1[:], 0.0, None, AL.is_ge)
            TS(u2[:], u1[:], 63.0, None, AL.is_le)
            TT(vy0[:], vy0[:], u2[:], AL.mult)
            TS(vy1[:], u1[:], -1.0, None, AL.is_ge)
            TS(u2[:], u1[:], 62.0, None, AL.is_le)
            TT(vy1[:], vy1[:], u2[:], AL.mult)

            TT(u0[:], ow_x[:], bw_x[:], AL.add)
            TS(u0[:], u0[:], -8.0, 72.0, AL.max, AL.min)
            floor_to(u1, fx, u0, ui, u2, vx0)
            TS(vx0[:], u1[:], 0.0, None, AL.is_ge)
            TS(u2[:], u1[:], 63.0, None, AL.is_le)
            TT(vx0[:], vx0[:], u2[:], AL.mult)
            TS(vx1[:], u1[:], -1.0, None, AL.is_ge)
            TS(u2[:], u1[:], 62.0, None, AL.is_le)
            TT(vx1[:], vx1[:], u2[:], AL.mult)
            TS(sel[:], u1[:], -1.0, None, AL.is_equal)

            TS(u0[:], fy[:], -1.0, 1.0, AL.mult, AL.add)
            TT(qy0[:], u0[:], vy0[:], AL.mult)
            TT(qy1[:], fy[:], vy1[:], AL.mult)
            TS(u0[:], fx[:], -1.0, 1.0, AL.mult, AL.add)
            TT(u0[:], u0[:], vx0[:], AL.mult)
            TT(u1[:], fx[:], vx1[:], AL.mult)
            TT(u2[:], u1[:], u0[:], AL.subtract)
            TT(u2[:], u2[:], sel[:], AL.mult)
            TT(wlx[:], u0[:], u2[:], AL.add)
            TT(u2[:], u1[:], sel[:], AL.mult)
            TT(wrx[:], u1[:], u2[:], AL.subtract)
            TT(wTL[:], qy0[:], wlx[:], AL.mult)
            TT(wTR[:], qy0[:], wrx[:], AL.mult)
            TT(wBL[:], qy1[:], wlx[:], AL.mult)
            vector.tensor_tensor(wBR[:], qy1[:], wrx[:], AL.mult)
            vector.drain().then_inc(s_wts, 1)

            # ---- diag weight builds (corners TL, TR; Act builds BL, BR) ----
            # dbuf[c%2][:, j, k, :] = diag(w_j[:, KK*c+k]) = id * w (scalar
            # per partition); the PE then applies the corner combine as
            # 4 accumulating matmuls g_tile @ diag per (k, cb) tile.
            for c in range(N_PB):
                if c >= 2:
                    vector.wait_ge(s_tr, 5 * (c - 1))
                d = dbuf[c % 2]
                if "combine" in skip:
                    vector.drain().then_inc(s_dg, 1)
                    continue
                for j2, w_ in ((0, wTL), (1, wTR), (2, wBL), (3, wBR)):
                    for k in range(KK):
                        vector.tensor_scalar(d[:, j2, k, :], id_sb[:],
                                             w_[:, KK * c + k:KK * c + k + 1],
                                             None, AL.mult)
                vector.drain().then_inc(s_dg, 1)

        @block.gpsimd
        def _(gpsimd):
            gpsimd.load_library(library_config.mlp)
            gpsimd.wait_ge(s_idx, 1)
            xt_view = xt[:].copy()
            xt_view.ap = bass_rust.VecI64Pair([[C, HW], [1, 2 * C]])
            for c in range(N_PB):
                if c >= NBUF:
                    gpsimd.wait_ge(s_tr, 5 * (c - NBUF + 1))
                i0 = c * 72
                # one dma_gather tops out at 1024 idx -> split 1152 = 1024 + 128
                for (lo, hi, s0, s1) in ((0, 64, 0, 8), (64, 72, 8, 9)):
                    nidx = (hi - lo) * 16
                    if "gather" in skip:
                        gpsimd.sem_inc(s_g[c % NBUF], 16)
                        gpsimd.sem_inc(s_g[NBUF + c % NBUF], 16)
                        continue
                    gpsimd.dma_gather(g_top[c % NBUF][:, s0:s1, :], xt_view,
                                      idx_top[:, i0 + lo:i0 + hi],
                                      nidx, nidx, 2 * C,
                                      elem_step=C).then_inc(s_g[c % NBUF], 16)
                    gpsimd.dma_gather(g_bot[c % NBUF][:, s0:s1, :], xt_view,
                                      idx_bot[:, i0 + lo:i0 + hi],
                                      nidx, nidx, 2 * C,
                                      elem_step=C).then_inc(s_g[NBUF + c % NBUF], 16)

        @block.tensor
        def _(tensor):
            tensor.wait_ge(s_io, IO_ALL)
            j = 0
            for c in range(N_PB):
                tensor.wait_ge(s_dg, c + 1)
                tensor.wait_ge(s_g[c % NBUF], 32 * (c // NBUF + 1))
                tensor.wait_ge(s_g[NBUF + c % NBUF], 32 * (c // NBUF + 1))
                gt, gb = g_top[c % NBUF], g_bot[c % NBUF]
                d = dbuf[c % 2]
                for k in range(KK):
                    for cb in range(2):
                        t = 2 * k + cb
                        g4, q = t // 4, t % 4
                        jj = c * 5 + g4          # global 4-group index (5 groups/chunk)
                        if "pe" in skip:
                            if q == 0:
                                tensor.sem_inc(s_tr, 1)
                            j += 1
                            continue
                        if q == 0 and jj >= NPST:
                            tensor.wait_ge(s_cpy, jj - NPST + 1)
                        corners = (
                            (gt[:, k, cb * 128:(cb + 1) * 128], d[:, 0, k, :]),
                            (gt[:, k, 256 + cb * 128:256 + (cb + 1) * 128],
                             d[:, 1, k, :]),
                            (gb[:, k, cb * 128:(cb + 1) * 128], d[:, 2, k, :]),
                            (gb[:, k, 256 + cb * 128:256 + (cb + 1) * 128],
                             d[:, 3, k, :]),
                        )
                        for ci_, (src, dd) in enumerate(corners):
                            mm = tensor.matmul(ps_t[jj % NPST][:, q, :], src, dd,
                                               start=(ci_ == 0), stop=(ci_ == 3))
                        if q == 3 or t == N_CKT - 1:
                            mm.then_inc(s_tr, 1)
                        j += 1
                if c % 4 == 3:
                    g = c // 4
                    tensor.wait_ge(s_cpy, 5 * (c + 1))
                    for ob in range(2):
                        if "pe" in skip:
                            tensor.sem_inc(s_mm, 1)
                            continue
                        if g >= 1:
                            tensor.wait_ge(s_oc, 2 * (g - 1) + ob + 1)
                        for t in range(N_CKT):
                            mm = tensor.matmul(
                                ps_g[ob][:],
                                wt_sb[:, t, ob * 128:(ob + 1) * 128],
                                cols[:, t, g * GEMM_NP:(g + 1) * GEMM_NP],
                                start=(t == 0), stop=(t == N_CKT - 1))
                        mm.then_inc(s_mm, 1)

        @block.scalar
        def _(scalar):
            j = 0
            for c in range(N_PB):
                for g4 in range(5):
                    nt = 4 if g4 < 4 else 2      # tiles in this group (18 = 4*4+2)
                    jj = c * 5 + g4
                    if "act" in skip:
                        scalar.sem_inc(s_cpy, 1)
                        continue
                    scalar.wait_ge(s_tr, jj + 1)
                    t0_ = g4 * 4
                    scalar.activation(cols[:, t0_:t0_ + nt, c * 128:(c + 1) * 128],
                                      ps_t[jj % NPST][:, 0:nt, :],
                                      AF.Copy).then_inc(s_cpy, 1)
                if c % 4 == 3:
                    g = c // 4
                    for ob in range(2):
                        jj = 2 * g + ob
                        if "act" in skip:
                            scalar.sem_inc(s_oc, 1)
                            continue
                        scalar.wait_ge(s_mm, jj + 1)
                        if g >= 1:
                            scalar.wait_ge(s_st[ob], 16 * g)
                        scalar.activation(out_sb[ob][:], ps_g[ob][:],
                                          AF.Copy).then_inc(s_oc, 1)

        blk_cm.__exit__(None, None, None)

    snap = nc._state.snapshot_sems()
    for rep in range(reps):
        emit_body()
        if rep < reps - 1:
            nc.clear_and_free_semaphores(nc._state.allocated_since(snap))
            nc.all_engine_barrier()
            nc._state.restore_sems(snap)

    st.close()
    nc.compile()
    return nc


def _host_prep(x, offset, weight):
    """Build the 8 per-core input maps."""
    f16 = np.float16
    # xT fp16, padded with one zero row: [HW+1, C]
    xts = []
    for b in range(B):
        xt = np.zeros(((HW + 1) * C,), dtype=f16)
        xt[:HW * C] = np.ascontiguousarray(
            x[b].reshape(C, HW).T).astype(f16).reshape(-1)
        xts.append(xt)
    # weights: wt[t, c, o] = weight[o, cb*128+c, ky, kx],  t = 2*(3*ky+kx) + cb
    wr = weight.reshape(O, C, KK).transpose(2, 1, 0)   # [KK, C, O]
    wt = np.empty((N_CKT, 128, O), dtype=f16)
    for k in range(KK):
        for cb in range(2):
            wt[2 * k + cb] = wr[k, cb * 128:(cb + 1) * 128, :].astype(f16)

    ident = np.eye(128, dtype=f16)

    ky, kx = np.meshgrid(np.arange(3), np.arange(3), indexing="ij")
    ky = ky.reshape(-1).astype(np.float32)   # [KK]
    kx = kx.reshape(-1).astype(np.float32)

    in_maps = []
    for core in range(N_CORES):
        b, hhalf = core // 2, core % 2
        i0 = hhalf * ROWS_PER_CORE
        off = offset[b].reshape(KK, 2, H, W)[:, :, i0:i0 + ROWS_PER_CORE, :]
        offy = off[:, 0].reshape(KK, P_CORE).astype(np.float32)   # [KK, P]
        offx = off[:, 1].reshape(KK, P_CORE).astype(np.float32)
        p = np.arange(P_CORE)
        basy = (i0 + p // W - 1).astype(np.float32)[None, :] + ky[:, None]  # [KK, P]
        basx = (p % W - 1).astype(np.float32)[None, :] + kx[:, None]

        # w-layout [128, N_SLOT]: (part, KK*pb + k) = val[k, pb*128 + part]
        def to_w(a):
            # a: [KK, P_CORE] -> [128, N_PB, KK] -> [128, N_SLOT]
            return np.ascontiguousarray(
                a.reshape(KK, N_PB, 128).transpose(2, 1, 0)).reshape(128, N_SLOT)

        # g-layout [128, NG]: (16g + l, 72*pb + 8*k + w) = val[k, pb*128 + w*16 + l]
        def to_g(a):
            a4 = a.reshape(KK, N_PB, 8, 16)          # [k, pb, w, l]
            g1 = a4.transpose(3, 1, 0, 2)            # [l, pb, k, w]
            g1 = np.ascontiguousarray(g1).reshape(16, N_IDX // 16)
            return np.tile(g1, (8, 1))               # replicate to 128 partitions

        in_maps.append({
            "xt": xts[b], "wt": wt, "ident": ident,
            "offy_w": to_w(offy), "offx_w": to_w(offx),
            "basy_w": to_w(np.broadcast_to(basy, offy.shape)),
            "basx_w": to_w(np.broadcast_to(basx, offx.shape)),
            "offy_g": to_g(offy), "offx_g": to_g(offx),
            "basy_g": to_g(np.broadcast_to(basy, offy.shape)),
            "basx_g": to_g(np.broadcast_to(basx, offx.shape)),
        })
    return in_maps


def _assemble(results):
    out = np.empty((B, O, H, W), dtype=np.float32)
    for core in range(N_CORES):
        b, hhalf = core // 2, core % 2
        i0 = hhalf * ROWS_PER_CORE
        out[b, :, i0:i0 + ROWS_PER_CORE, :] = \
            np.asarray(results[core]["out"]).reshape(O, ROWS_PER_CORE, W)
    return out


def _make_exec(nc, donate=False):
    """Build a cached jitted SPMD executor for a compiled Bass module.

    Replicates concourse.bass2jax.run_bass_via_pjrt's lowering (same
    _bass_exec_p bind / shard_map layout) but returns a reusable jitted
    callable, so repeated invocations skip re-trace/re-lower/re-compile.
    """
    import jax
    import numpy as _np
    from jax.sharding import Mesh, PartitionSpec
    from jax.experimental.shard_map import shard_map
    from concourse import bass2jax
    import concourse.mybir as mybir

    bass2jax.install_neuronx_cc_hook()
    assert nc.dbg_addr is None
    partition_name = (nc.partition_id_tensor.name
                      if nc.partition_id_tensor else None)

    in_names, out_names, out_avals, zero_outs = [], [], [], []
    for alloc in nc.m.functions[0].allocations:
        if not isinstance(alloc, mybir.MemoryLocationSet):
            continue
        name = alloc.memorylocations[0].name
        if alloc.kind == "ExternalInput":
            if name != partition_name:
                in_names.append(name)
        elif alloc.kind == "ExternalOutput":
            out_names.append(name)
            shape = tuple(alloc.tensor_shape)
            dtype = mybir.dt.np(alloc.dtype)
            out_avals.append(jax.core.ShapedArray(shape, dtype))
            zero_outs.append(_np.zeros(shape, dtype))
    n_params = len(in_names)
    all_names = list(in_names) + list(out_names)
    if partition_name is not None:
        all_names.append(partition_name)
    all_names = tuple(all_names)

    def _body(*args):
        operands = list(args)
        if partition_name is not None:
            operands.append(bass2jax.partition_id_tensor())
        outs = bass2jax._bass_exec_p.bind(
            *operands,
            out_avals=tuple(out_avals),
            in_names=all_names,
            out_names=tuple(out_names),
            lowering_input_output_aliases=(),
            sim_require_finite=True,
            sim_require_nnan=True,
            nc=nc,
        )
        return tuple(outs)

    devices = jax.devices()[:N_CORES]
    mesh = Mesh(np.asarray(devices), ("core",))
    n_out = len(out_names)
    fn = jax.jit(
        shard_map(_body, mesh=mesh,
                  in_specs=(PartitionSpec("core"),) * (n_params + n_out),
                  out_specs=(PartitionSpec("core"),) * n_out,
                  check_rep=False),
        donate_argnums=tuple(range(n_params, n_params + n_out)) if donate else (),
        keep_unused=True,
    )
    return {"fn": fn, "in_names": in_names, "out_names": out_names,
            "zero_outs": zero_outs, "mesh": mesh, "n_params": n_params}


def _concat_inputs(ex, in_maps):
    return [np.concatenate([in_maps[c][n] for c in range(N_CORES)], axis=0)
            for n in ex["in_names"]]


def _concat_zeros(ex):
    return [np.zeros((N_CORES * z.shape[0], *z.shape[1:]), z.dtype)
            for z in ex["zero_outs"]]


def kernel(x, offset, weight):
    import jax
    x = np.asarray(x, dtype=np.float32)
    offset = np.asarray(offset, dtype=np.float32)
    weight = np.asarray(weight, dtype=np.float32)
    if "nc" not in _CACHE:
        _CACHE["nc"] = _build_nc()
    if "exec" not in _CACHE:
        _CACHE["exec"] = _make_exec(_CACHE["nc"])
    ex = _CACHE["exec"]
    in_maps = _host_prep(x, offset, weight)
    outs = ex["fn"](*_concat_inputs(ex, in_maps), *_concat_zeros(ex))
    full = np.asarray(outs[0]).reshape(N_CORES, O, P_CORE)
    results = [{"out": full[c]} for c in range(N_CORES)]
    return _assemble(results)



# revision 30
# speedup vs baseline: 1.3859x; 1.2305x over previous
"""Deformable conv (B=4, C=256, H=W=64, O=256, K=3, pad=1) on 8 NeuronCores.

Sharding: core = (image b, row-half h): each core computes out[b, :, h*32:(h+1)*32, :].

Per-core device pipeline:
  1. DVE computes gather indices (int16, wrapped-16 layout) and bilinear corner
     weights (f32, per-(pos,tap) scalars) from the raw conv offsets.
  2. GpSimd dma_gathers fp16 channel-pair rows (512ch covering x0,x0+1) from a
     host-transposed xT [HW, C] fp16 image in DRAM: 2 gathers (top/bottom row)
     per (tap, position); 1KB descriptors; NBUF=3-deep buffer rotation.
  3. DVE (corners TL/TR/BL) and Act (corner BR) expand the per-(pos,tap)
     corner weights into dense diagonal matrices d = id * w.
  4. PE fuses transpose + 4-corner bilinear combine: per (tap, ch-block) tile,
     4 accumulating fp16 matmuls g_tile[pos, ch] @ diag(w_corner) land the
     weighted [ch, pos] tile in fp32 PSUM; Act copies tiles into the fp16
     cols buffer; PE then runs the O=256 x CK=2304 x P=2048 GEMM with fp32
     PSUM accumulation.

Execution: _make_exec builds a cached jitted shard_map executor (same
_bass_exec_p lowering as bass2jax.run_bass_via_pjrt) so repeated calls skip
re-trace/re-lower/re-compile.
"""

import numpy as np

B, C, H, W = 4, 256, 64, 64
O, KK = 256, 9
HW = H * W                      # 4096
ROWS_PER_CORE = H // 2          # 32
P_CORE = ROWS_PER_CORE * W      # 2048 output positions per core
N_PB = P_CORE // 128            # 16 chunks (position blocks)
IDX_PER_CHUNK = KK * 128        # 1152
N_IDX = N_PB * IDX_PER_CHUNK    # 18432
N_SLOT = N_PB * KK              # 144
N_CKT = 2 * KK                  # 18 ck-tiles of 128
GEMM_NP = 512                   # positions per GEMM pass
N_GP = P_CORE // GEMM_NP        # 4 GEMM passes
N_CORES = 8

_CACHE = {}


def _build_nc(reps=1, skip=()):
    skip = set(skip)
    import concourse.bacc as bacc
    import concourse.mybir as mybir
    from concourse import library_config
    import bass_rust

    F32, F16, I16, I32 = (mybir.dt.float32, mybir.dt.float16,
                          mybir.dt.int16, mybir.dt.int32)
    AF = mybir.ActivationFunctionType
    AL = mybir.AluOpType

    nc = bacc.Bacc("TRN2", num_swdge_queues=4)

    # ---- DRAM parameters (per-core inputs) ----
    xt = nc.declare_dram_parameter("xt", [(HW + 1) * C], F16, isOutput=False)
    wt = nc.declare_dram_parameter("wt", [N_CKT, 128, O], F16, isOutput=False)
    offy_w = nc.declare_dram_parameter("offy_w", [128, N_SLOT], F32, isOutput=False)
    offx_w = nc.declare_dram_parameter("offx_w", [128, N_SLOT], F32, isOutput=False)
    basy_w = nc.declare_dram_parameter("basy_w", [128, N_SLOT], F32, isOutput=False)
    basx_w = nc.declare_dram_parameter("basx_w", [128, N_SLOT], F32, isOutput=False)
    offy_g = nc.declare_dram_parameter("offy_g", [128, N_IDX // 16], F32, isOutput=False)
    offx_g = nc.declare_dram_parameter("offx_g", [128, N_IDX // 16], F32, isOutput=False)
    basy_g = nc.declare_dram_parameter("basy_g", [128, N_IDX // 16], F32, isOutput=False)
    basx_g = nc.declare_dram_parameter("basx_g", [128, N_IDX // 16], F32, isOutput=False)
    ident = nc.declare_dram_parameter("ident", [128, 128], F16, isOutput=False)
    out = nc.declare_dram_parameter("out", [O, P_CORE], F32, isOutput=True)

    NG = N_IDX // 16     # 1152 free-dim of g-layout

    from contextlib import ExitStack
    st = ExitStack()
    sb = lambda n, s, d: st.enter_context(nc.sbuf_tensor(n, s, d))
    ps = lambda n, s, d: st.enter_context(nc.psum_tensor(n, s, d))

    # ---- SBUF ----
    idx_top = sb("idx_top", [128, NG], I16)
    idx_bot = sb("idx_bot", [128, NG], I16)
    # w-layout pipeline
    ow_y = sb("ow_y", [128, N_SLOT], F32)
    ow_x = sb("ow_x", [128, N_SLOT], F32)
    bw_y = sb("bw_y", [128, N_SLOT], F32)
    bw_x = sb("bw_x", [128, N_SLOT], F32)
    u0 = sb("u0", [128, N_SLOT], F32)
    u1 = sb("u1", [128, N_SLOT], F32)
    u2 = sb("u2", [128, N_SLOT], F32)
    ui = sb("ui", [128, N_SLOT], I32)
    fy = sb("fy", [128, N_SLOT], F32)
    fx = sb("fx", [128, N_SLOT], F32)
    vy0 = sb("vy0", [128, N_SLOT], F32)
    vy1 = sb("vy1", [128, N_SLOT], F32)
    vx0 = sb("vx0", [128, N_SLOT], F32)
    vx1 = sb("vx1", [128, N_SLOT], F32)
    sel = sb("sel", [128, N_SLOT], F32)
    qy0 = sb("qy0", [128, N_SLOT], F32)
    qy1 = sb("qy1", [128, N_SLOT], F32)
    wlx = sb("wlx", [128, N_SLOT], F32)
    wrx = sb("wrx", [128, N_SLOT], F32)
    wTL = sb("wTL", [128, N_SLOT], F32)
    wTR = sb("wTR", [128, N_SLOT], F32)
    wBL = sb("wBL", [128, N_SLOT], F32)
    wBR = sb("wBR", [128, N_SLOT], F32)
    # transposed cols, full [128, N_CKT, P_CORE] fp16
    cols = sb("cols", [128, N_CKT, P_CORE], F16)
    # weights in SBUF [128, N_CKT, O] fp16
    wt_sb = sb("wt_sb", [128, N_CKT, O], F16)
    id_sb = sb("id_sb", [128, 128], F16)
    out_sb = [sb(f"out_sb{i}", [128, GEMM_NP], F32) for i in range(2)]

    # g-layout coord pipeline: temps are dead once idx_top/idx_bot are
    # written, so they live in a nested stack (top of the SBUF allocation
    # stack) that is closed before the gather buffers are allocated — the
    # allocator reuses the space. Runtime-safe: the first gather write is
    # gated on s_idx, which fires after the last coord-temp read.
    st_coord = ExitStack()
    sbc = lambda n, s, d: st_coord.enter_context(nc.sbuf_tensor(n, s, d))
    og_y = sbc("og_y", [128, NG], F32)
    og_x = sbc("og_x", [128, NG], F32)
    bg_y = sbc("bg_y", [128, NG], F32)
    bg_x = sbc("bg_x", [128, NG], F32)
    t0 = sbc("t0", [128, NG], F32)
    t1 = sbc("t1", [128, NG], F32)
    t2 = sbc("t2", [128, NG], F32)
    ti = sbc("ti", [128, NG], I32)
    s_clip = sbc("s_clip", [128, NG], F32)
    st_coord.close()

    # gather buffers (NBUF-deep): [128, KK, 512] fp16 — reuse coord space
    NBUF = 3
    g_top = [sb(f"g_top{i}", [128, KK, 512], F16) for i in range(NBUF)]
    g_bot = [sb(f"g_bot{i}", [128, KK, 512], F16) for i in range(NBUF)]
    # per-(corner, tap) diagonal weight matrices, double buffered:
    # dbuf[b][:, corner, k, :] = diag(w_corner[:, KK*c+k]) as dense [128,128]
    dbuf = [sb(f"dbuf{i}", [128, 4, KK, 128], F16) for i in range(2)]

    NPST = 3
    ps_t = [ps(f"ps_t{i}", [128, 4, 128], F32) for i in range(NPST)]
    ps_g = [ps(f"ps_g{i}", [128, GEMM_NP], F32) for i in range(2)]

    N_LOADS = 10
    IO_ALL = N_LOADS * 16

    def emit_body():
        s_io = nc.alloc_semaphore("s_io")
        s_idx = nc.alloc_semaphore("s_idx")
        s_wts = nc.alloc_semaphore("s_wts")
        s_g = [nc.alloc_semaphore(f"s_g{i}") for i in range(2 * NBUF)]
        s_dg = nc.alloc_semaphore("s_dg")
        s_tr = nc.alloc_semaphore("s_tr")
        s_cpy = nc.alloc_semaphore("s_cpy")
        s_mm = nc.alloc_semaphore("s_mm")
        s_oc = nc.alloc_semaphore("s_oc")
        s_st = [nc.alloc_semaphore(f"s_st{i}") for i in range(2)]
        blk_cm = nc.Block()
        block = blk_cm.__enter__()

        @block.sync
        def _(sync):
            sync.dma_start(wt_sb[:], wt.rearrange("t c e -> c t e")).then_inc(s_io, 16)
            for src, dst in [(offy_w, ow_y), (offx_w, ow_x), (basy_w, bw_y),
                             (basx_w, bw_x), (offy_g, og_y), (offx_g, og_x),
                             (basy_g, bg_y), (basx_g, bg_x), (ident, id_sb)]:
                sync.dma_start(dst[:], src[:]).then_inc(s_io, 16)
            for g in range(N_GP):
                for ob in range(2):
                    j = 2 * g + ob
                    sync.wait_ge(s_oc, j + 1)
                    sync.dma_start(out[ob * 128:(ob + 1) * 128,
                                       g * GEMM_NP:(g + 1) * GEMM_NP],
                                   out_sb[ob][:]).then_inc(s_st[ob], 16)
            sync.wait_ge(s_st[0], 16 * N_GP)
            sync.wait_ge(s_st[1], 16 * N_GP)

        @block.vector
        def _(vector):
            vector.wait_ge(s_io, IO_ALL)

            def TT(out_, a, b, op):
                vector.tensor_tensor(out_, a, b, op)
                vector.drain()

            def TS(out_, a, s1, s2, op0, op1=None):
                if op1 is None:
                    vector.tensor_scalar(out_, a, s1, None, op0)
                else:
                    vector.tensor_scalar(out_, a, s1, s2, op0, op1)
                vector.drain()

            def CP(out_, a):
                vector.tensor_copy(out_, a)
                vector.drain()

            def STT(out_, a, sc, b, op0, op1):
                vector.scalar_tensor_tensor(out_, a, sc, b, op0, op1)
                vector.drain()

            def floor_to(dst_f, dst_frac, src, tmp_i, tmp_a, tmp_b):
                # dst_f = floor(src); dst_frac = src - floor(src)
                CP(tmp_i[:], src[:])                 # rint cast
                CP(tmp_a[:], tmp_i[:])               # back to f32
                TT(tmp_b[:], tmp_a[:], src[:], AL.is_gt)
                TT(dst_f[:], tmp_a[:], tmp_b[:], AL.subtract)
                if dst_frac is not None:
                    TT(dst_frac[:], src[:], dst_f[:], AL.subtract)

            # ---- idx computation (g layout) ----
            TT(t0[:], og_y[:], bg_y[:], AL.add)
            TS(t0[:], t0[:], -8.0, 72.0, AL.max, AL.min)
            floor_to(t1, None, t0, ti, t2, s_clip)
            TT(t0[:], og_x[:], bg_x[:], AL.add)
            TS(t0[:], t0[:], -8.0, 72.0, AL.max, AL.min)
            floor_to(t2, None, t0, ti, s_clip, og_x)   # og_x clobbered as tmp
            TS(s_clip[:], t2[:], 0.0, 63.0, AL.max, AL.min)
            TS(t0[:], t1[:], 0.0, 63.0, AL.max, AL.min)
            STT(t0[:], t0[:], 64.0, s_clip[:], AL.mult, AL.add)
            CP(idx_top[:], t0[:])
            TS(t0[:], t1[:], 1.0, 0.0, AL.add, AL.max)
            TS(t0[:], t0[:], 63.0, None, AL.min)
            STT(t0[:], t0[:], 64.0, s_clip[:], AL.mult, AL.add)
            vector.tensor_copy(idx_bot[:], t0[:])
            vector.drain().then_inc(s_idx, 1)

            # ---- weight computation (w layout) ----
            TT(u0[:], ow_y[:], bw_y[:], AL.add)
            TS(u0[:], u0[:], -8.0, 72.0, AL.max, AL.min)
            floor_to(u1, fy, u0, ui, u2, vy0)
            TS(vy0[:], u1[:], 0.0, None, AL.is_ge)
            TS(u2[:], u1[:], 63.0, None, AL.is_le)
            TT(vy0[:], vy0[:], u2[:], AL.mult)
            TS(vy1[:], u1[:], -1.0, None, AL.is_ge)
            TS(u2[:], u1[:], 62.0, None, AL.is_le)
            TT(vy1[:], vy1[:], u2[:], AL.mult)

            TT(u0[:], ow_x[:], bw_x[:], AL.add)
            TS(u0[:], u0[:], -8.0, 72.0, AL.max, AL.min)
            floor_to(u1, fx, u0, ui, u2, vx0)
            TS(vx0[:], u1[:], 0.0, None, AL.is_ge)
            TS(u2[:], u1[:], 63.0, None, AL.is_le)
            TT(vx0[:], vx0[:], u2[:], AL.mult)
            TS(vx1[:], u1[:], -1.0, None, AL.is_ge)
            TS(u2[:], u1[:], 62.0, None, AL.is_le)
            TT(vx1[:], vx1[:], u2[:], AL.mult)
            TS(sel[:], u1[:], -1.0, None, AL.is_equal)

            TS(u0[:], fy[:], -1.0, 1.0, AL.mult, AL.add)
            TT(qy0[:], u0[:], vy0[:], AL.mult)
            TT(qy1[:], fy[:], vy1[:], AL.mult)
            TS(u0[:], fx[:], -1.0, 1.0, AL.mult, AL.add)
            TT(u0[:], u0[:], vx0[:], AL.mult)
            TT(u1[:], fx[:], vx1[:], AL.mult)
            TT(u2[:], u1[:], u0[:], AL.subtract)
            TT(u2[:], u2[:], sel[:], AL.mult)
            TT(wlx[:], u0[:], u2[:], AL.add)
            TT(u2[:], u1[:], sel[:], AL.mult)
            TT(wrx[:], u1[:], u2[:], AL.subtract)
            TT(wTL[:], qy0[:], wlx[:], AL.mult)
            TT(wTR[:], qy0[:], wrx[:], AL.mult)
            TT(wBL[:], qy1[:], wlx[:], AL.mult)
            vector.tensor_tensor(wBR[:], qy1[:], wrx[:], AL.mult)
            vector.drain().then_inc(s_wts, 1)

            # ---- diag weight builds (corners TL, TR; Act builds BL, BR) ----
            # dbuf[c%2][:, j, k, :] = diag(w_j[:, KK*c+k]) = id * w (scalar
            # per partition); the PE then applies the corner combine as
            # 4 accumulating matmuls g_tile @ diag per (k, cb) tile.
            for c in range(N_PB):
                if c >= 2:
                    vector.wait_ge(s_tr, 5 * (c - 1))
                d = dbuf[c % 2]
                if "combine" in skip:
                    vector.drain().then_inc(s_dg, 1)
                    continue
                for j2, w_ in ((0, wTL), (1, wTR), (2, wBL), (3, wBR)):
                    for k in range(KK):
                        vector.tensor_scalar(d[:, j2, k, :], id_sb[:],
                                             w_[:, KK * c + k:KK * c + k + 1],
                                             None, AL.mult)
                vector.drain().then_inc(s_dg, 1)

        @block.gpsimd
        def _(gpsimd):
            gpsimd.load_library(library_config.mlp)
            gpsimd.wait_ge(s_idx, 1)
            xt_view = xt[:].copy()
            xt_view.ap = bass_rust.VecI64Pair([[C, HW], [1, 2 * C]])
            for c in range(N_PB):
                if c >= NBUF:
                    gpsimd.wait_ge(s_tr, 5 * (c - NBUF + 1))
                i0 = c * 72
                # one dma_gather tops out at 1024 idx -> split 1152 = 1024 + 128
                for qi, (lo, hi, s0, s1) in enumerate(((0, 64, 0, 8),
                                                      (64, 72, 8, 9))):
                    nidx = (hi - lo) * 16
                    if "gather" in skip:
                        gpsimd.sem_inc(s_g[c % NBUF], 16)
                        gpsimd.sem_inc(s_g[NBUF + c % NBUF], 16)
                        continue
                    gpsimd.dma_gather(g_top[c % NBUF][:, s0:s1, :], xt_view,
                                      idx_top[:, i0 + lo:i0 + hi],
                                      nidx, nidx, 2 * C, elem_step=C,
                                      queue_num=2 * qi).then_inc(
                                          s_g[c % NBUF], 16)
                    gpsimd.dma_gather(g_bot[c % NBUF][:, s0:s1, :], xt_view,
                                      idx_bot[:, i0 + lo:i0 + hi],
                                      nidx, nidx, 2 * C, elem_step=C,
                                      queue_num=2 * qi + 1).then_inc(
                                          s_g[NBUF + c % NBUF], 16)

        @block.tensor
        def _(tensor):
            tensor.wait_ge(s_io, IO_ALL)
            j = 0
            for c in range(N_PB):
                tensor.wait_ge(s_dg, c + 1)
                tensor.wait_ge(s_g[c % NBUF], 32 * (c // NBUF + 1))
                tensor.wait_ge(s_g[NBUF + c % NBUF], 32 * (c // NBUF + 1))
                gt, gb = g_top[c % NBUF], g_bot[c % NBUF]
                d = dbuf[c % 2]
                for k in range(KK):
                    for cb in range(2):
                        t = 2 * k + cb
                        g4, q = t // 4, t % 4
                        jj = c * 5 + g4          # global 4-group index (5 groups/chunk)
                        if "pe" in skip:
                            if q == 0:
                                tensor.sem_inc(s_tr, 1)
                            j += 1
                            continue
                        if q == 0 and jj >= NPST:
                            tensor.wait_ge(s_cpy, jj - NPST + 1)
                        corners = (
                            (gt[:, k, cb * 128:(cb + 1) * 128], d[:, 0, k, :]),
                            (gt[:, k, 256 + cb * 128:256 + (cb + 1) * 128],
                             d[:, 1, k, :]),
                            (gb[:, k, cb * 128:(cb + 1) * 128], d[:, 2, k, :]),
                            (gb[:, k, 256 + cb * 128:256 + (cb + 1) * 128],
                             d[:, 3, k, :]),
                        )
                        for ci_, (src, dd) in enumerate(corners):
                            mm = tensor.matmul(ps_t[jj % NPST][:, q, :], src, dd,
                                               start=(ci_ == 0), stop=(ci_ == 3))
                        if q == 3 or t == N_CKT - 1:
                            mm.then_inc(s_tr, 1)
                        j += 1
                if c % 4 == 3:
                    g = c // 4
                    tensor.wait_ge(s_cpy, 5 * (c + 1))
                    for ob in range(2):
                        if "pe" in skip:
                            tensor.sem_inc(s_mm, 1)
                            continue
                        if g >= 1:
                            tensor.wait_ge(s_oc, 2 * (g - 1) + ob + 1)
                        for t in range(N_CKT):
                            mm = tensor.matmul(
                                ps_g[ob][:],
                                wt_sb[:, t, ob * 128:(ob + 1) * 128],
                                cols[:, t, g * GEMM_NP:(g + 1) * GEMM_NP],
                                start=(t == 0), stop=(t == N_CKT - 1))
                        mm.then_inc(s_mm, 1)

        @block.scalar
        def _(scalar):
            j = 0
            for c in range(N_PB):
                for g4 in range(5):
                    nt = 4 if g4 < 4 else 2      # tiles in this group (18 = 4*4+2)
                    jj = c * 5 + g4
                    if "act" in skip:
                        scalar.sem_inc(s_cpy, 1)
                        continue
                    scalar.wait_ge(s_tr, jj + 1)
                    t0_ = g4 * 4
                    scalar.activation(cols[:, t0_:t0_ + nt, c * 128:(c + 1) * 128],
                                      ps_t[jj % NPST][:, 0:nt, :],
                                      AF.Copy).then_inc(s_cpy, 1)
                if c % 4 == 3:
                    g = c // 4
                    for ob in range(2):
                        jj = 2 * g + ob
                        if "act" in skip:
                            scalar.sem_inc(s_oc, 1)
                            continue
                        scalar.wait_ge(s_mm, jj + 1)
                        if g >= 1:
                            scalar.wait_ge(s_st[ob], 16 * g)
                        scalar.activation(out_sb[ob][:], ps_g[ob][:],
                                          AF.Copy).then_inc(s_oc, 1)

        blk_cm.__exit__(None, None, None)

    snap = nc._state.snapshot_sems()
    for rep in range(reps):
        emit_body()
        if rep < reps - 1:
            nc.clear_and_free_semaphores(nc._state.allocated_since(snap))
            nc.all_engine_barrier()
            nc._state.restore_sems(snap)

    st.close()
    nc.compile()
    return nc


def _host_prep(x, offset, weight):
    """Build the 8 per-core input maps."""
    f16 = np.float16
    # xT fp16, padded with one zero row: [HW+1, C]
    xts = []
    for b in range(B):
        xt = np.zeros(((HW + 1) * C,), dtype=f16)
        xt[:HW * C] = np.ascontiguousarray(
            x[b].reshape(C, HW).T).astype(f16).reshape(-1)
        xts.append(xt)
    # weights: wt[t, c, o] = weight[o, cb*128+c, ky, kx],  t = 2*(3*ky+kx) + cb
    wr = weight.reshape(O, C, KK).transpose(2, 1, 0)   # [KK, C, O]
    wt = np.empty((N_CKT, 128, O), dtype=f16)
    for k in range(KK):
        for cb in range(2):
            wt[2 * k + cb] = wr[k, cb * 128:(cb + 1) * 128, :].astype(f16)

    ident = np.eye(128, dtype=f16)

    ky, kx = np.meshgrid(np.arange(3), np.arange(3), indexing="ij")
    ky = ky.reshape(-1).astype(np.float32)   # [KK]
    kx = kx.reshape(-1).astype(np.float32)

    in_maps = []
    for core in range(N_CORES):
        b, hhalf = core // 2, core % 2
        i0 = hhalf * ROWS_PER_CORE
        off = offset[b].reshape(KK, 2, H, W)[:, :, i0:i0 + ROWS_PER_CORE, :]
        offy = off[:, 0].reshape(KK, P_CORE).astype(np.float32)   # [KK, P]
        offx = off[:, 1].reshape(KK, P_CORE).astype(np.float32)
        p = np.arange(P_CORE)
        basy = (i0 + p // W - 1).astype(np.float32)[None, :] + ky[:, None]  # [KK, P]
        basx = (p % W - 1).astype(np.float32)[None, :] + kx[:, None]

        # w-layout [128, N_SLOT]: (part, KK*pb + k) = val[k, pb*128 + part]
        def to_w(a):
            # a: [KK, P_CORE] -> [128, N_PB, KK] -> [128, N_SLOT]
            return np.ascontiguousarray(
                a.reshape(KK, N_PB, 128).transpose(2, 1, 0)).reshape(128, N_SLOT)

        # g-layout [128, NG]: (16g + l, 72*pb + 8*k + w) = val[k, pb*128 + w*16 + l]
        def to_g(a):
            a4 = a.reshape(KK, N_PB, 8, 16)          # [k, pb, w, l]
            g1 = a4.transpose(3, 1, 0, 2)            # [l, pb, k, w]
            g1 = np.ascontiguousarray(g1).reshape(16, N_IDX // 16)
            return np.tile(g1, (8, 1))               # replicate to 128 partitions

        in_maps.append({
            "xt": xts[b], "wt": wt, "ident": ident,
            "offy_w": to_w(offy), "offx_w": to_w(offx),
            "basy_w": to_w(np.broadcast_to(basy, offy.shape)),
            "basx_w": to_w(np.broadcast_to(basx, offx.shape)),
            "offy_g": to_g(offy), "offx_g": to_g(offx),
            "basy_g": to_g(np.broadcast_to(basy, offy.shape)),
            "basx_g": to_g(np.broadcast_to(basx, offx.shape)),
        })
    return in_maps


def _assemble(results):
    out = np.empty((B, O, H, W), dtype=np.float32)
    for core in range(N_CORES):
        b, hhalf = core // 2, core % 2
        i0 = hhalf * ROWS_PER_CORE
        out[b, :, i0:i0 + ROWS_PER_CORE, :] = \
            np.asarray(results[core]["out"]).reshape(O, ROWS_PER_CORE, W)
    return out


def _make_exec(nc, donate=False):
    """Build a cached jitted SPMD executor for a compiled Bass module.

    Replicates concourse.bass2jax.run_bass_via_pjrt's lowering (same
    _bass_exec_p bind / shard_map layout) but returns a reusable jitted
    callable, so repeated invocations skip re-trace/re-lower/re-compile.
    """
    import jax
    import numpy as _np
    from jax.sharding import Mesh, PartitionSpec
    from jax.experimental.shard_map import shard_map
    from concourse import bass2jax
    import concourse.mybir as mybir

    bass2jax.install_neuronx_cc_hook()
    assert nc.dbg_addr is None
    partition_name = (nc.partition_id_tensor.name
                      if nc.partition_id_tensor else None)

    in_names, out_names, out_avals, zero_outs = [], [], [], []
    for alloc in nc.m.functions[0].allocations:
        if not isinstance(alloc, mybir.MemoryLocationSet):
            continue
        name = alloc.memorylocations[0].name
        if alloc.kind == "ExternalInput":
            if name != partition_name:
                in_names.append(name)
        elif alloc.kind == "ExternalOutput":
            out_names.append(name)
            shape = tuple(alloc.tensor_shape)
            dtype = mybir.dt.np(alloc.dtype)
            out_avals.append(jax.core.ShapedArray(shape, dtype))
            zero_outs.append(_np.zeros(shape, dtype))
    n_params = len(in_names)
    all_names = list(in_names) + list(out_names)
    if partition_name is not None:
        all_names.append(partition_name)
    all_names = tuple(all_names)

    def _body(*args):
        operands = list(args)
        if partition_name is not None:
            operands.append(bass2jax.partition_id_tensor())
        outs = bass2jax._bass_exec_p.bind(
            *operands,
            out_avals=tuple(out_avals),
            in_names=all_names,
            out_names=tuple(out_names),
            lowering_input_output_aliases=(),
            sim_require_finite=True,
            sim_require_nnan=True,
            nc=nc,
        )
        return tuple(outs)

    devices = jax.devices()[:N_CORES]
    mesh = Mesh(np.asarray(devices), ("core",))
    n_out = len(out_names)
    fn = jax.jit(
        shard_map(_body, mesh=mesh,
                  in_specs=(PartitionSpec("core"),) * (n_params + n_out),
                  out_specs=(PartitionSpec("core"),) * n_out,
                  check_rep=False),
        donate_argnums=tuple(range(n_params, n_params + n_out)) if donate else (),
        keep_unused=True,
    )
    return {"fn": fn, "in_names": in_names, "out_names": out_names,
            "zero_outs": zero_outs, "mesh": mesh, "n_params": n_params}


def _concat_inputs(ex, in_maps):
    return [np.concatenate([in_maps[c][n] for c in range(N_CORES)], axis=0)
            for n in ex["in_names"]]


def _concat_zeros(ex):
    return [np.zeros((N_CORES * z.shape[0], *z.shape[1:]), z.dtype)
            for z in ex["zero_outs"]]


def kernel(x, offset, weight):
    import jax
    x = np.asarray(x, dtype=np.float32)
    offset = np.asarray(offset, dtype=np.float32)
    weight = np.asarray(weight, dtype=np.float32)
    if "nc" not in _CACHE:
        _CACHE["nc"] = _build_nc()
    if "exec" not in _CACHE:
        _CACHE["exec"] = _make_exec(_CACHE["nc"])
    ex = _CACHE["exec"]
    in_maps = _host_prep(x, offset, weight)
    outs = ex["fn"](*_concat_inputs(ex, in_maps), *_concat_zeros(ex))
    full = np.asarray(outs[0]).reshape(N_CORES, O, P_CORE)
    results = [{"out": full[c]} for c in range(N_CORES)]
    return _assemble(results)

